# revision 1
# baseline (speedup 1.0000x reference)
"""Trainium2 Bass kernel for a 2-layer GCN encoder + MLP head (PyG GCNConv).

Strategy (8 NeuronCores, node-parallel):
  - Nodes sharded by contiguous range: core q owns rows [q*SH, (q+1)*SH).
  - conv linear (x @ Wc1) computed shard-local on PE (bf16 inputs, fp32 PSUM).
  - z0 shards AllGather'ed to a full replicated table Z0 [NP, 256] (bf16).
  - Aggregation out[d] = sum_e norm_e * z[src_e] done per destination tile:
    edges grouped (host-side bin-packing) into T tiles of <=128 dst nodes and
    <=C*128 edges; per tile one dma_gather fetches all edge source rows, a
    fused DVE tensor_scalar builds the norm-scaled one-hot S_T [128e,128d]
    per 128-edge chunk, and PE matmuls accumulate S_T.T @ msg in PSUM.
  - dma_gather indices are int16 (max 32767) so the Z table is split in two
    halves and each tile issues two gathers (lo/hi src).
  - h1 tiles are bias+relu'ed, transposed on PE, pushed through Wc2, and the
    z1 rows are indirect-DMA-scattered back to the shard layout; AllGather;
    layer-2 aggregation likewise produces h2; AllGather; the MLP head runs
    on B/8 variants per core (feature-major matmuls, ACT bias+relu).
All heavy compute is bf16 with fp32 PSUM accumulation.
"""
import sys

for _p in ("/opt/trn_rl_repo",):
    if _p not in sys.path:
        sys.path.insert(0, _p)

import numpy as np
import ml_dtypes

bf16 = ml_dtypes.bfloat16

P = 128
H = 256          # gcn hidden width (fixed)
HH = 128         # head hidden width (fixed)
OH = 40          # wt_onehot + mut_onehot width (fixed)
NCORES = 8


class Cfg:
    def __init__(self, N, E, D_IN, B):
        self.N, self.E, self.D_IN, self.B = N, E, D_IN, B
        assert N % NCORES == 0
        self.SH = N // NCORES                      # real rows per shard
        shp = -(-self.SH // P) * P
        if shp == self.SH:
            shp += P                               # need >=1 dump row
        self.SHP = shp                             # padded rows per shard
        self.NP = NCORES * self.SHP                # padded global rows
        assert self.NP % 2 == 0
        self.NPH = self.NP // 2                    # half-table rows (int16 idx)
        assert self.NPH < 32768
        self.KT = -(-D_IN // P)                    # k tiles for conv1
        self.KPAD = self.KT * P
        self.MT = self.SHP // P                    # m tiles per shard
        self.BPC = self.B // NCORES                # batch per core
        assert self.BPC % P == 0
        self.BCH = self.BPC // P                   # batch chunks


REAL = Cfg(N=50000, E=800000, D_IN=1281, B=4096)


# ---------------------------------------------------------------- host prep

def _pack_idx16(seq):
    """idx sequence [n] -> wrapped-16 + replicated layout [128, n//16] int16."""
    n = seq.shape[0]
    assert n % 16 == 0
    a = seq.reshape(n // 16, 16).T.astype(np.int16)
    return np.tile(a, (8, 1))


def _pack_core(cfg, cl, ch, d_loc, srcp, nv, count_only=False):
    """Bin-pack one core's edges into tiles (<=128 nodes, <=cl*128 lo edges,
    <=ch*128 hi edges). Returns tiles as (nodes, lo_lists, hi_lists) or count."""
    SH = cfg.SH
    order = np.argsort(d_loc, kind="stable")
    d_s = d_loc[order]
    counts = np.bincount(d_s, minlength=SH)
    starts = np.zeros(SH + 1, np.int64)
    np.cumsum(counts, out=starts[1:])
    lo_mask = srcp[order] < cfg.NPH
    # per-node lo/hi counts
    klo = np.zeros(SH, np.int64)
    np.add.at(klo, d_s[lo_mask], 1)
    ktot = counts
    khi = ktot - klo

    node_order = np.argsort(-ktot, kind="stable")
    cap_l, cap_h = cl * P, ch * P
    tiles = []  # [n_nodes, lo_cnt, hi_cnt, node_list]
    for r in node_order:
        kl, kh = klo[r], khi[r]
        placed = False
        for t in tiles:
            if t[0] < P and t[1] + kl <= cap_l and t[2] + kh <= cap_h:
                t[0] += 1
                t[1] += kl
                t[2] += kh
                t[3].append(r)
                placed = True
                break
        if not placed:
            tiles.append([1, kl, kh, [r]])
    if count_only:
        return len(tiles)
    return tiles, order, starts, lo_mask


def _build_core_arrays(cfg, q, T, cl, ch, tiles, order, starts, lo_mask,
                       srcp, nv):
    """Build gidx/dsel/nrm/scat arrays for one core."""
    C = cl + ch
    n_slots = T * C * P
    gidx_seq = np.zeros(n_slots, np.int64)
    dsel = np.zeros((P, T * C), np.float32)
    nrm = np.zeros((P, T * C), np.float32)
    scat = np.full((P, T), cfg.SH, np.int32)  # dump row default

    for t, tl in enumerate(tiles):
        lo_idx, lo_d, lo_n = [], [], []
        hi_idx, hi_d, hi_n = [], [], []
        for d, r in enumerate(tl[3]):
            scat[d, t] = r
            es = order[starts[r]:starts[r + 1]]
            lm = lo_mask[starts[r]:starts[r + 1]]
            sp = srcp[es]
            nn = nv[es]
            lo_idx.extend(sp[lm].tolist())
            lo_d.extend([d] * int(lm.sum()))
            lo_n.extend(nn[lm].tolist())
            hm = ~lm
            hi_idx.extend((sp[hm] - cfg.NPH).tolist())
            hi_d.extend([d] * int(hm.sum()))
            hi_n.extend(nn[hm].tolist())
        npad_l = cl * P - len(lo_idx)
        npad_h = ch * P - len(hi_idx)
        assert npad_l >= 0 and npad_h >= 0
        seq_idx = lo_idx + [0] * npad_l + hi_idx + [0] * npad_h
        seq_d = lo_d + [0] * npad_l + hi_d + [0] * npad_h
        seq_n = lo_n + [0.0] * npad_l + hi_n + [0.0] * npad_h
        base = t * C * P
        gidx_seq[base:base + C * P] = seq_idx
        a_d = np.asarray(seq_d, np.float32).reshape(C, P).T
        a_n = np.asarray(seq_n, np.float32).reshape(C, P).T
        dsel[:, t * C:(t + 1) * C] = a_d
        nrm[:, t * C:(t + 1) * C] = a_n

    # pack idx per (tile, half) segment
    cols = []
    for t in range(T):
        base = t * C * P
        cols.append(_pack_idx16(gidx_seq[base:base + cl * P]))
        cols.append(_pack_idx16(gidx_seq[base + cl * P:base + C * P]))
    gidx = np.concatenate(cols, axis=1)  # [128, T*C*8]
    return gidx, dsel, nrm, scat


def host_prep(cfg, x, wt_onehot, mut_onehot, Wc1, bc1, Wc2, bc2,
              Wh1, bh1, Wh2, bh2, Wh3, bh3, edge_index, var_node_idx):
    N, E, SH, SHP = cfg.N, cfg.E, cfg.SH, cfg.SHP
    src = np.asarray(edge_index[0], np.int64)
    dst = np.asarray(edge_index[1], np.int64)
    loop = np.arange(N, dtype=np.int64)
    src_all = np.concatenate([src, loop])
    dst_all = np.concatenate([dst, loop])
    deg = np.bincount(dst_all, minlength=N).astype(np.float32)
    dinv = np.where(deg > 0, 1.0 / np.sqrt(np.maximum(deg, 1.0)), 0.0).astype(np.float32)
    norm = (dinv[src_all] * dinv[dst_all]).astype(np.float32)
    srcp_all = (src_all // SH) * SHP + (src_all % SH)

    core_of = dst_all // SH
    per_core = []
    for q in range(NCORES):
        m = core_of == q
        per_core.append((dst_all[m] - q * SH, srcp_all[m], norm[m]))

    # choose caps
    avg_deg = (E + N) / N
    base = max(1, int(np.ceil(avg_deg * P / 2 / P)))
    cands = [(base, base), (base + 1, base + 1), (base, base + 1),
             (base + 1, base), (base + 2, base + 2)]
    best = None
    packs_cache = {}
    for (cl, ch) in cands:
        Ts = []
        packs = []
        for q in range(NCORES):
            d_loc, srcp, nv = per_core[q]
            pk = _pack_core(cfg, cl, ch, d_loc, srcp, nv)
            packs.append(pk)
            Ts.append(len(pk[0]))
        T_need = max(Ts)
        cost = T_need * (cl + ch)
        if best is None or cost < best[0]:
            best = (cost, cl, ch, T_need)
            packs_cache = {q: packs[q] for q in range(NCORES)}
    _, cl, ch, T = best
    C = cl + ch

    # shared weights
    wc1 = np.zeros((cfg.KPAD, H), bf16)
    wc1[:cfg.D_IN] = np.asarray(Wc1, np.float32).astype(bf16)
    wc2 = np.asarray(Wc2, np.float32).astype(bf16)
    wh1 = np.zeros((3 * P, HH), bf16)
    wh1[:H + OH] = np.asarray(Wh1, np.float32).astype(bf16)
    wh2 = np.asarray(Wh2, np.float32).astype(bf16)
    wh3 = np.asarray(Wh3, np.float32).astype(bf16)
    bb1 = np.tile(np.asarray(bc1, np.float32)[None, :], (P, 1))
    bb2 = np.tile(np.asarray(bc2, np.float32)[None, :], (P, 1))
    bh1v = np.asarray(bh1, np.float32).reshape(HH, 1)
    bh2v = np.asarray(bh2, np.float32).reshape(HH // 2, 1)
    bh3v = np.asarray(bh3, np.float32).reshape(1, 1)

    x = np.asarray(x, np.float32)
    wt_b = np.asarray(wt_onehot, np.float32).astype(bf16)
    mut_b = np.asarray(mut_onehot, np.float32).astype(bf16)
    vni = np.asarray(var_node_idx, np.int64)
    vrow = (vni // SH) * SHP + (vni % SH)

    in_maps = []
    meta = dict(T=T, cl=cl, ch=ch)
    for q in range(NCORES):
        d_loc, srcp, nv = per_core[q]
        tiles, order, starts, lo_mask = packs_cache[q]
        gidx, dsel, nrm, scat = _build_core_arrays(
            cfg, q, T, cl, ch, tiles, order, starts, lo_mask, srcp, nv)
        xT = np.zeros((cfg.KPAD, SHP), bf16)
        xT[:cfg.D_IN, :SH] = x[q * SH:(q + 1) * SH].T.astype(bf16)
        vr = vrow[q * cfg.BPC:(q + 1) * cfg.BPC]
        vidx = vr.reshape(cfg.BCH, P).T.astype(np.int32)
        ohT = np.concatenate(
            [wt_b[q * cfg.BPC:(q + 1) * cfg.BPC].T,
             mut_b[q * cfg.BPC:(q + 1) * cfg.BPC].T], axis=0)  # [40, BPC]
        in_maps.append(dict(
            xT=xT, gidx=gidx, dsel=dsel, nrm=nrm, scat=scat,
            vidx=np.ascontiguousarray(vidx), ohT=np.ascontiguousarray(ohT),
            wc1=wc1, wc2=wc2, wh1=wh1, wh2=wh2, wh3=wh3,
            bb1=bb1, bb2=bb2, bh1v=bh1v, bh2v=bh2v, bh3v=bh3v,
        ))
    return in_maps, meta


# ------------------------------------------------------------- bass program

def build_program(cfg, T, cl, ch):
    import concourse.bass as bass
    import concourse.mybir as mybir
    import concourse.tile as tile
    from concourse import bacc
    from concourse.masks import make_identity

    C = cl + ch
    nc = bacc.Bacc("TRN2", target_bir_lowering=False, debug=False,
                   num_devices=NCORES)
    f32, bfl, i16, i32 = (mybir.dt.float32, mybir.dt.bfloat16,
                          mybir.dt.int16, mybir.dt.int32)

    # I/O
    xT = nc.dram_tensor("xT", [cfg.KPAD, cfg.SHP], bfl, kind="ExternalInput")
    gidx = nc.dram_tensor("gidx", [P, T * C * 8], i16, kind="ExternalInput")
    dsel = nc.dram_tensor("dsel", [P, T * C], f32, kind="ExternalInput")
    nrm = nc.dram_tensor("nrm", [P, T * C], f32, kind="ExternalInput")
    scat = nc.dram_tensor("scat", [P, T], i32, kind="ExternalInput")
    vidx = nc.dram_tensor("vidx", [P, cfg.BCH], i32, kind="ExternalInput")
    ohT = nc.dram_tensor("ohT", [OH, cfg.BPC], bfl, kind="ExternalInput")
    wc1 = nc.dram_tensor("wc1", [cfg.KPAD, H], bfl, kind="ExternalInput")
    wc2 = nc.dram_tensor("wc2", [H, H], bfl, kind="ExternalInput")
    wh1 = nc.dram_tensor("wh1", [3 * P, HH], bfl, kind="ExternalInput")
    wh2 = nc.dram_tensor("wh2", [HH, HH // 2], bfl, kind="ExternalInput")
    wh3 = nc.dram_tensor("wh3", [HH // 2, 1], bfl, kind="ExternalInput")
    bb1 = nc.dram_tensor("bb1", [P, H], f32, kind="ExternalInput")
    bb2 = nc.dram_tensor("bb2", [P, H], f32, kind="ExternalInput")
    bh1v = nc.dram_tensor("bh1v", [HH, 1], f32, kind="ExternalInput")
    bh2v = nc.dram_tensor("bh2v", [HH // 2, 1], f32, kind="ExternalInput")
    bh3v = nc.dram_tensor("bh3v", [1, 1], f32, kind="ExternalInput")
    out = nc.dram_tensor("out", [1, cfg.BPC], f32, kind="ExternalOutput")

    # internal DRAM
    z0in = nc.dram_tensor("z0in", [cfg.SHP, H], bfl, kind="Internal")
    z1in = nc.dram_tensor("z1in", [cfg.SHP, H], bfl, kind="Internal")
    h2in = nc.dram_tensor("h2in", [cfg.SHP, H], bfl, kind="Internal")
    Z0 = nc.dram_tensor("Z0", [cfg.NP, H], bfl, kind="Internal",
                        addr_space="Shared")
    Z1 = nc.dram_tensor("Z1", [cfg.NP, H], bfl, kind="Internal",
                        addr_space="Shared")
    H2 = nc.dram_tensor("H2", [cfg.NP, H], bfl, kind="Internal",
                        addr_space="Shared")
    rg = [list(range(NCORES))]

    with tile.TileContext(nc) as tc:
        with tc.tile_pool(name="const", bufs=1) as const:
            iota_i = const.tile([P, P], i32)
            nc.gpsimd.iota(iota_i[:], pattern=[[1, P]], base=0,
                           channel_multiplier=0)
            iota_b = const.tile([P, P], bfl)
            nc.vector.tensor_copy(iota_b[:], iota_i[:])
            ident = const.tile([P, P], bfl)
            make_identity(nc, ident[:])

            def load(ap, shape, dt):
                t = const.tile(shape, dt, tag=ap.tensor.name)
                nc.sync.dma_start(t[:], ap)
                return t

            wc1_sb = load(wc1.rearrange("(t p) n -> p t n", p=P)[:],
                          [P, cfg.KT, H], bfl)
            wc2_sb = load(wc2.rearrange("(t p) n -> p t n", p=P)[:],
                          [P, 2, H], bfl)
            wh1_sb = load(wh1.rearrange("(t p) n -> p t n", p=P)[:],
                          [P, 3, HH], bfl)
            wh2_sb = load(wh2[:], [HH, HH // 2], bfl)
            wh3_sb = load(wh3[:], [HH // 2, 1], bfl)
            bb1_sb = load(bb1[:], [P, H], f32)
            bb2_sb = load(bb2[:], [P, H], f32)
            bh1_sb = load(bh1v[:], [HH, 1], f32)
            bh2_sb = load(bh2v[:], [HH // 2, 1], f32)
            bh3_sb = load(bh3v[:], [1, 1], f32)
            gidx_sb = load(gidx[:], [P, T * C * 8], i16)
            dsel_sb = load(dsel[:], [P, T * C], f32)
            nrm_sb = load(nrm[:], [P, T * C], f32)
            scat_sb = load(scat[:], [P, T], i32)
            vidx_sb = load(vidx[:], [P, cfg.BCH], i32)
            ohT_sb = load(ohT[:], [OH, cfg.BPC], bfl)

            # zero the dump rows of the scatter targets
            zpad = const.tile([P, H], bfl)
            nc.any.memset(zpad[:], 0.0)
            npad = cfg.SHP - cfg.SH
            nc.sync.dma_start(z1in[cfg.SH:cfg.SHP, :], zpad[:npad, :])
            nc.sync.dma_start(h2in[cfg.SH:cfg.SHP, :], zpad[:npad, :])

            # ---------------- phase A: conv1 linear z0 = x @ Wc1
            MBS = 7
            with tc.tile_pool(name="c1sb", bufs=3) as c1sb, \
                 tc.tile_pool(name="c1ev", bufs=3) as c1ev, \
                 tc.tile_pool(name="c1ps", bufs=MBS + 1, space="PSUM") as c1ps:
                for mb0 in range(0, cfg.MT, MBS):
                    mbn = min(MBS, cfg.MT - mb0)
                    accs = [c1ps.tile([P, H], f32, tag="convacc",
                                      name=f"convacc_{mb0}_{j}")
                            for j in range(mbn)]
                    for kt in range(cfg.KT):
                        slab = c1sb.tile([P, MBS * P], bfl, tag="slab")
                        nc.sync.dma_start(
                            slab[:, :mbn * P],
                            xT[kt * P:(kt + 1) * P, mb0 * P:(mb0 + mbn) * P])
                        for j in range(mbn):
                            nc.tensor.matmul(
                                accs[j][:], lhsT=slab[:, j * P:(j + 1) * P],
                                rhs=wc1_sb[:, kt, :],
                                start=(kt == 0), stop=(kt == cfg.KT - 1))
                    for j in range(mbn):
                        zb = c1ev.tile([P, H], bfl, tag="zev")
                        nc.vector.tensor_copy(zb[:], accs[j][:])
                        r0 = (mb0 + j) * P
                        nc.sync.dma_start(z0in[r0:r0 + P, :], zb[:])

            nc.gpsimd.collective_compute(
                "AllGather", mybir.AluOpType.bypass, replica_groups=rg,
                ins=[z0in[:]], outs=[Z0[:]])

            # ---------------- aggregation layers
            def agg_layer(Z, bias_sb, out_dram, do_conv2):
                with tc.tile_pool(name="agsb", bufs=3) as agsb, \
                     tc.tile_pool(name="agps", bufs=2, space="PSUM") as agps:
                    Zlo = Z[:cfg.NPH, :]
                    Zhi = Z[cfg.NPH:, :]
                    for t in range(T):
                        msg = agsb.tile([P, C, H], bfl, tag="msg")
                        off = t * C * 8
                        nc.gpsimd.dma_gather(
                            msg[:, :cl, :], Zlo, gidx_sb[:, off:off + cl * 8],
                            cl * P, cl * P, H, single_packet=False)
                        nc.gpsimd.dma_gather(
                            msg[:, cl:, :], Zhi,
                            gidx_sb[:, off + cl * 8:off + C * 8],
                            ch * P, ch * P, H, single_packet=False)
                        acc = agps.tile([P, H], f32, tag="agacc")
                        for c in range(C):
                            st = agsb.tile([P, P], bfl, tag="st")
                            col = t * C + c
                            nc.vector.tensor_scalar(
                                out=st[:], in0=iota_b[:],
                                scalar1=dsel_sb[:, col:col + 1],
                                scalar2=nrm_sb[:, col:col + 1],
                                op0=mybir.AluOpType.is_equal,
                                op1=mybir.AluOpType.mult)
                            nc.tensor.matmul(acc[:], lhsT=st[:],
                                             rhs=msg[:, c, :],
                                             start=(c == 0), stop=(c == C - 1))
                        hf = agsb.tile([P, H], f32, tag="hf")
                        nc.vector.tensor_tensor(out=hf[:], in0=acc[:],
                                                in1=bias_sb[:],
                                                op=mybir.AluOpType.add)
                        hb = agsb.tile([P, H], bfl, tag="hb")
                        nc.vector.tensor_scalar_max(hb[:], hf[:], 0.0)
                        if do_conv2:
                            ht = agsb.tile([P, H], bfl, tag="ht")
                            for k in range(2):
                                pt = agps.tile([P, P], bfl, space="PSUM",
                                               tag="pt")
                                nc.tensor.transpose(
                                    pt[:], hb[:, k * P:(k + 1) * P], ident[:])
                                nc.vector.tensor_copy(
                                    ht[:, k * P:(k + 1) * P], pt[:])
                            pz = agps.tile([P, H], f32, tag="pz")
                            for k in range(2):
                                nc.tensor.matmul(
                                    pz[:], lhsT=ht[:, k * P:(k + 1) * P],
                                    rhs=wc2_sb[:, k, :],
                                    start=(k == 0), stop=(k == 1))
                            res = agsb.tile([P, H], bfl, tag="res")
                            nc.vector.tensor_copy(res[:], pz[:])
                        else:
                            res = hb
                        nc.gpsimd.indirect_dma_start(
                            out=out_dram[:],
                            out_offset=bass.IndirectOffsetOnAxis(
                                ap=scat_sb[:, t:t + 1], axis=0),
                            in_=res[:], in_offset=None)

            agg_layer(Z0, bb1_sb, z1in, do_conv2=True)
            nc.gpsimd.collective_compute(
                "AllGather", mybir.AluOpType.bypass, replica_groups=rg,
                ins=[z1in[:]], outs=[Z1[:]])
            agg_layer(Z1, bb2_sb, h2in, do_conv2=False)
            nc.gpsimd.collective_compute(
                "AllGather", mybir.AluOpType.bypass, replica_groups=rg,
                ins=[h2in[:]], outs=[H2[:]])

            # ---------------- head
            with tc.tile_pool(name="hdsb", bufs=2) as hdsb, \
                 tc.tile_pool(name="hdps", bufs=2, space="PSUM") as hdps:
                zt0 = hdsb.tile([P, cfg.BPC], bfl, tag="zt0")
                zt1 = hdsb.tile([P, cfg.BPC], bfl, tag="zt1")
                for j in range(cfg.BCH):
                    g = hdsb.tile([P, H], bfl, tag="hg")
                    nc.gpsimd.indirect_dma_start(
                        out=g[:], out_offset=None, in_=H2[:],
                        in_offset=bass.IndirectOffsetOnAxis(
                            ap=vidx_sb[:, j:j + 1], axis=0))
                    for k in range(2):
                        pt = hdps.tile([P, P], bfl, space="PSUM", tag="hpt")
                        nc.tensor.transpose(pt[:], g[:, k * P:(k + 1) * P],
                                            ident[:])
                        dstt = zt0 if k == 0 else zt1
                        nc.vector.tensor_copy(
                            dstt[:, j * P:(j + 1) * P], pt[:])
                ph1 = hdps.tile([P, cfg.BPC], f32, tag="ph1")
                nc.tensor.matmul(ph1[:], lhsT=wh1_sb[:, 0, :], rhs=zt0[:],
                                 start=True, stop=False)
                nc.tensor.matmul(ph1[:], lhsT=wh1_sb[:, 1, :], rhs=zt1[:],
                                 start=False, stop=False)
                nc.tensor.matmul(ph1[:], lhsT=wh1_sb[:OH, 2, :],
                                 rhs=ohT_sb[:], start=False, stop=True)
                a1 = hdsb.tile([P, cfg.BPC], bfl, tag="a1")
                nc.scalar.activation(a1[:], ph1[:],
                                     mybir.ActivationFunctionType.Relu,
                                     bias=bh1_sb[:])
                ph2 = hdps.tile([HH // 2, cfg.BPC], f32, tag="ph2")
                nc.tensor.matmul(ph2[:], lhsT=wh2_sb[:], rhs=a1[:],
                                 start=True, stop=True)
                a2 = hdsb.tile([HH // 2, cfg.BPC], bfl, tag="a2")
                nc.scalar.activation(a2[:], ph2[:],
                                     mybir.ActivationFunctionType.Relu,
                                     bias=bh2_sb[:])
                ph3 = hdps.tile([1, cfg.BPC], f32, tag="ph3")
                nc.tensor.matmul(ph3[:], lhsT=wh3_sb[:], rhs=a2[:],
                                 start=True, stop=True)
                osb = hdsb.tile([1, cfg.BPC], f32, tag="osb")
                nc.vector.tensor_scalar_add(osb[:], ph3[:], bh3_sb[:, :1])
                nc.sync.dma_start(out[:], osb[:])

    nc.compile()
    return nc


# ------------------------------------------------------------------ driver

_CACHE = {}


def _get_program(cfg, T, cl, ch):
    key = (cfg.N, cfg.E, cfg.D_IN, cfg.B, T, cl, ch)
    if key not in _CACHE:
        _CACHE[key] = build_program(cfg, T, cl, ch)
    return _CACHE[key]


def kernel(**inputs):
    cfg = REAL
    in_maps, meta = host_prep(cfg, **inputs)
    nc = _get_program(cfg, meta["T"], meta["cl"], meta["ch"])
    from concourse import bass_utils
    res = bass_utils.run_bass_kernel_spmd(
        nc, in_maps, core_ids=list(range(NCORES)))
    outs = [np.asarray(res.results[q]["out"]).reshape(cfg.BPC)
            for q in range(NCORES)]
    return np.concatenate(outs).astype(np.float32)



# revision 2
# speedup vs baseline: 1.2263x; 1.2263x over previous
"""Trainium2 Bass kernel for a 2-layer GCN encoder + MLP head (PyG GCNConv).

v2 strategy (8 NeuronCores, node-parallel):
  - Nodes sharded by contiguous range; dst tiles are contiguous 128-node
    blocks (T = ceil(SH/128)), so aggregation output writes are plain
    contiguous DMA (no indirect scatter).
  - norm = dinv[src]*dinv[dst] is factored: tables store dinv[i]*z[i]
    (scaled at conv evacuation); aggregation post-multiplies by dinv[d]
    fused with relu on DVE; bias enters via a 1-row matmul (sqdeg x b).
  - Self-loop edges form chunk 0 of each tile, loaded contiguously from the
    core-local table shard (z0in/z1in) with a plain dma_start.
  - Remaining edges: per tile two dma_gathers (lo/hi int16 index halves)
    with per-tile real counts (16-padded) and round-robin SWDGE queues
    (4 queues) to avoid descriptor-ring serialization.
  - One-hot routing matrices built on DVE with a single broadcast-AP
    is_equal per tile; aggregation = per-chunk one-hot matmuls in PSUM.
  - conv2 fused in the layer-1 tile loop via PE transposes.
All heavy compute bf16 with fp32 PSUM accumulation.
"""
import sys

for _p in ("/opt/trn_rl_repo",):
    if _p not in sys.path:
        sys.path.insert(0, _p)

import numpy as np
import ml_dtypes

bf16 = ml_dtypes.bfloat16

P = 128
H = 256          # gcn hidden width (fixed)
HH = 128         # head hidden width (fixed)
OH = 40          # wt_onehot + mut_onehot width (fixed)
NCORES = 8
NQ = 4           # SWDGE queues


class Cfg:
    def __init__(self, N, E, D_IN, B):
        self.N, self.E, self.D_IN, self.B = N, E, D_IN, B
        assert N % NCORES == 0
        self.SH = N // NCORES                      # real rows per shard
        self.T = -(-self.SH // P)                  # dst tiles per shard
        shp = max(self.T * P, -(-self.SH // P) * P)
        if shp == self.SH:
            shp += P
        self.SHP = -(-shp // P) * P                # padded rows per shard
        self.NP = NCORES * self.SHP                # padded global rows
        assert self.NP % 2 == 0
        self.NPH = self.NP // 2                    # half-table rows (int16 idx)
        assert self.NPH < 32768
        self.KT = -(-D_IN // P)                    # k tiles for conv1
        self.KPAD = self.KT * P
        self.MT = self.SHP // P                    # m tiles per shard
        self.BPC = self.B // NCORES                # batch per core
        assert self.BPC % P == 0
        self.BCH = self.BPC // P                   # batch chunks


REAL = Cfg(N=50000, E=800000, D_IN=1281, B=4096)


# ---------------------------------------------------------------- host prep

def _pack_idx16(seq):
    """idx sequence [n] (n%16==0) -> wrapped-16 replicated [128, n//16] i16."""
    n = seq.shape[0]
    assert n % 16 == 0
    a = seq.reshape(n // 16, 16).T.astype(np.int16)
    return np.tile(a, (8, 1))


def host_prep(cfg, x, wt_onehot, mut_onehot, Wc1, bc1, Wc2, bc2,
              Wh1, bh1, Wh2, bh2, Wh3, bh3, edge_index, var_node_idx):
    N, E, SH, SHP, T = cfg.N, cfg.E, cfg.SH, cfg.SHP, cfg.T
    src = np.asarray(edge_index[0], np.int64)
    dst = np.asarray(edge_index[1], np.int64)
    # degree includes self loops (dst counts + 1)
    deg = (np.bincount(dst, minlength=N) + 1).astype(np.float32)
    dinv = (1.0 / np.sqrt(deg)).astype(np.float32)
    sqdeg = np.sqrt(deg).astype(np.float32)
    srcp = (src // SH) * SHP + (src % SH)

    # drop self-referencing edges? (none in random data, but (i,i) entries in
    # edge_index are real edges distinct from the implicit self loop)
    core_of = dst // SH

    # per-core, per-tile edge lists
    per_core = []
    CLmax = CHmax = 0
    for q in range(NCORES):
        m = core_of == q
        d_loc = dst[m] - q * SH
        sp = srcp[m]
        tile_of = d_loc // P
        order = np.argsort(tile_of, kind="stable")
        d_s, sp_s, t_s = d_loc[order], sp[order], tile_of[order]
        tstarts = np.searchsorted(t_s, np.arange(T + 1))
        tiles = []
        for t in range(T):
            a, b = tstarts[t], tstarts[t + 1]
            spt, dt_ = sp_s[a:b], d_s[a:b] - t * P
            lo = spt < cfg.NPH
            tiles.append(((spt[lo], dt_[lo]), (spt[~lo] - cfg.NPH, dt_[~lo])))
            CLmax = max(CLmax, -(-len(dt_[lo]) // P))
            CHmax = max(CHmax, -(-(len(dt_) - lo.sum()) // P))
        per_core.append(tiles)
    CL, CH = int(CLmax), int(CHmax)
    C = 1 + CL + CH   # self chunk + lo chunks + hi chunks

    # per-tile static counts must be IDENTICAL across cores (one program):
    # use the max over cores for each tile's lo/hi counts.
    nlo = np.zeros(T, np.int64)
    nhi = np.zeros(T, np.int64)
    for q in range(NCORES):
        for t in range(T):
            (sl, _), (sh_, _) = per_core[q][t]
            nlo[t] = max(nlo[t], len(sl))
            nhi[t] = max(nhi[t], len(sh_))
    nlo16 = ((nlo + 15) // 16) * 16
    nhi16 = ((nhi + 15) // 16) * 16
    clo = np.maximum(1, -(-nlo16 // P))  # chunks per tile (>=1 for layout)
    chi = np.maximum(1, -(-nhi16 // P))
    # column offsets into gidx (units of 16-idx columns)
    lo_off = np.zeros(T + 1, np.int64)
    hi_off = np.zeros(T + 1, np.int64)
    np.cumsum(nlo16 // 16, out=lo_off[1:])
    hi_base = lo_off[T]
    np.cumsum(nhi16 // 16, out=hi_off[1:])
    gcols = int(lo_off[T] + hi_off[T])

    meta = dict(CL=CL, CH=CH, C=C,
                nlo16=tuple(int(v) for v in nlo16),
                nhi16=tuple(int(v) for v in nhi16),
                clo=tuple(int(v) for v in clo),
                chi=tuple(int(v) for v in chi),
                gcols=gcols, hi_base=int(hi_base))

    # shared weights
    wc1 = np.zeros((cfg.KPAD, H), bf16)
    wc1[:cfg.D_IN] = np.asarray(Wc1, np.float32).astype(bf16)
    wc2 = np.asarray(Wc2, np.float32).astype(bf16)
    wh1 = np.zeros((3 * P, HH), bf16)
    wh1[:H + OH] = np.asarray(Wh1, np.float32).astype(bf16)
    wh2 = np.asarray(Wh2, np.float32).astype(bf16)
    wh3 = np.asarray(Wh3, np.float32).astype(bf16)
    b1row = np.asarray(bc1, np.float32).astype(bf16).reshape(1, H)
    b2row = np.asarray(bc2, np.float32).astype(bf16).reshape(1, H)
    bh1v = np.asarray(bh1, np.float32).reshape(HH, 1)
    bh2v = np.asarray(bh2, np.float32).reshape(HH // 2, 1)
    bh3v = np.asarray(bh3, np.float32).reshape(1, 1)

    x = np.asarray(x, np.float32)
    wt_b = np.asarray(wt_onehot, np.float32).astype(bf16)
    mut_b = np.asarray(mut_onehot, np.float32).astype(bf16)
    vni = np.asarray(var_node_idx, np.int64)
    vrow = (vni // SH) * SHP + (vni % SH)

    in_maps = []
    for q in range(NCORES):
        gidx_seq = np.zeros(gcols * 16, np.int64)
        dsel = np.full((P, T * C), 999.0, np.float32)
        for t in range(T):
            (sl, dl), (sh_, dh_) = per_core[q][t]
            # self chunk col: diagonal for real rows
            nreal = min(P, SH - t * P)
            dsel[:nreal, t * C] = np.arange(nreal, dtype=np.float32)
            base = lo_off[t] * 16
            gidx_seq[base:base + len(sl)] = sl
            a_d = np.full(clo[t] * P, 999.0, np.float32)
            a_d[:len(dl)] = dl
            dsel[:, t * C + 1:t * C + 1 + clo[t]] = \
                a_d.reshape(clo[t], P).T
            base = (hi_base + hi_off[t]) * 16
            gidx_seq[base:base + len(sh_)] = sh_
            a_d = np.full(chi[t] * P, 999.0, np.float32)
            a_d[:len(dh_)] = dh_
            dsel[:, t * C + 1 + clo[t]:t * C + 1 + clo[t] + chi[t]] = \
                a_d.reshape(chi[t], P).T
        gidx = _pack_idx16(gidx_seq)

        # dinv per m-tile column [128, MT]; sqdeg row [1, T*P]
        dloc = np.zeros(SHP, np.float32)
        dloc[:SH] = dinv[q * SH:(q + 1) * SH]
        dinv_tbl = dloc.reshape(cfg.MT, P).T.copy()
        sq = np.zeros(T * P, np.float32)
        sq[:SH] = sqdeg[q * SH:(q + 1) * SH]
        sqrow = sq.reshape(1, T * P).astype(bf16)

        xT = np.zeros((cfg.KPAD, SHP), bf16)
        xT[:cfg.D_IN, :SH] = x[q * SH:(q + 1) * SH].T.astype(bf16)
        vr = vrow[q * cfg.BPC:(q + 1) * cfg.BPC]
        vidx = vr.reshape(cfg.BCH, P).T.astype(np.int32)
        ohT = np.concatenate(
            [wt_b[q * cfg.BPC:(q + 1) * cfg.BPC].T,
             mut_b[q * cfg.BPC:(q + 1) * cfg.BPC].T], axis=0)  # [40, BPC]
        in_maps.append(dict(
            xT=xT, gidx=gidx, dsel=dsel.astype(bf16),
            dinv_tbl=np.ascontiguousarray(dinv_tbl),
            sqrow=np.ascontiguousarray(sqrow),
            vidx=np.ascontiguousarray(vidx), ohT=np.ascontiguousarray(ohT),
            wc1=wc1, wc2=wc2, wh1=wh1, wh2=wh2, wh3=wh3,
            b1row=b1row, b2row=b2row,
            bh1v=bh1v, bh2v=bh2v, bh3v=bh3v,
        ))
    return in_maps, meta


# ------------------------------------------------------------- bass program

def build_program(cfg, meta):
    import concourse.bass as bass
    import concourse.mybir as mybir
    import concourse.tile as tile
    from concourse import bacc
    from concourse.masks import make_identity

    T = cfg.T
    CL, CH, C = meta["CL"], meta["CH"], meta["C"]
    nlo16, nhi16 = meta["nlo16"], meta["nhi16"]
    clo, chi = meta["clo"], meta["chi"]
    gcols, hi_base = meta["gcols"], meta["hi_base"]

    nc = bacc.Bacc("TRN2", target_bir_lowering=False, debug=False,
                   num_devices=NCORES, num_swdge_queues=NQ)
    f32, bfl, i16, i32 = (mybir.dt.float32, mybir.dt.bfloat16,
                          mybir.dt.int16, mybir.dt.int32)

    # I/O
    xT = nc.dram_tensor("xT", [cfg.KPAD, cfg.SHP], bfl, kind="ExternalInput")
    gidx = nc.dram_tensor("gidx", [P, gcols], i16, kind="ExternalInput")
    dsel = nc.dram_tensor("dsel", [P, T * C], bfl, kind="ExternalInput")
    dinv_tbl = nc.dram_tensor("dinv_tbl", [P, cfg.MT], f32,
                              kind="ExternalInput")
    sqrow = nc.dram_tensor("sqrow", [1, T * P], bfl, kind="ExternalInput")
    vidx = nc.dram_tensor("vidx", [P, cfg.BCH], i32, kind="ExternalInput")
    ohT = nc.dram_tensor("ohT", [OH, cfg.BPC], bfl, kind="ExternalInput")
    wc1 = nc.dram_tensor("wc1", [cfg.KPAD, H], bfl, kind="ExternalInput")
    wc2 = nc.dram_tensor("wc2", [H, H], bfl, kind="ExternalInput")
    wh1 = nc.dram_tensor("wh1", [3 * P, HH], bfl, kind="ExternalInput")
    wh2 = nc.dram_tensor("wh2", [HH, HH // 2], bfl, kind="ExternalInput")
    wh3 = nc.dram_tensor("wh3", [HH // 2, 1], bfl, kind="ExternalInput")
    b1row = nc.dram_tensor("b1row", [1, H], bfl, kind="ExternalInput")
    b2row = nc.dram_tensor("b2row", [1, H], bfl, kind="ExternalInput")
    bh1v = nc.dram_tensor("bh1v", [HH, 1], f32, kind="ExternalInput")
    bh2v = nc.dram_tensor("bh2v", [HH // 2, 1], f32, kind="ExternalInput")
    bh3v = nc.dram_tensor("bh3v", [1, 1], f32, kind="ExternalInput")
    out = nc.dram_tensor("out", [1, cfg.BPC], f32, kind="ExternalOutput")

    # internal DRAM
    z0in = nc.dram_tensor("z0in", [cfg.SHP, H], bfl, kind="Internal")
    z1in = nc.dram_tensor("z1in", [cfg.SHP, H], bfl, kind="Internal")
    h2in = nc.dram_tensor("h2in", [cfg.SHP, H], bfl, kind="Internal")
    Z0 = nc.dram_tensor("Z0", [cfg.NP, H], bfl, kind="Internal",
                        addr_space="Shared")
    Z1 = nc.dram_tensor("Z1", [cfg.NP, H], bfl, kind="Internal",
                        addr_space="Shared")
    H2 = nc.dram_tensor("H2", [cfg.NP, H], bfl, kind="Internal",
                        addr_space="Shared")
    rg = [list(range(NCORES))]

    with tile.TileContext(nc) as tc:
        with tc.tile_pool(name="const", bufs=1) as const:
            iota_i = const.tile([P, P], i32)
            nc.gpsimd.iota(iota_i[:], pattern=[[1, P]], base=0,
                           channel_multiplier=0)
            iota_b = const.tile([P, P], bfl)
            nc.vector.tensor_copy(iota_b[:], iota_i[:])
            ident = const.tile([P, P], bfl)
            make_identity(nc, ident[:])

            def load(ap, shape, dt):
                t = const.tile(shape, dt, tag=ap.tensor.name)
                nc.sync.dma_start(t[:], ap)
                return t

            wc1_sb = load(wc1.rearrange("(t p) n -> p t n", p=P)[:],
                          [P, cfg.KT, H], bfl)
            wc2_sb = load(wc2.rearrange("(t p) n -> p t n", p=P)[:],
                          [P, 2, H], bfl)
            wh1_sb = load(wh1.rearrange("(t p) n -> p t n", p=P)[:],
                          [P, 3, HH], bfl)
            wh2_sb = load(wh2[:], [HH, HH // 2], bfl)
            wh3_sb = load(wh3[:], [HH // 2, 1], bfl)
            b1_sb = load(b1row[:], [1, H], bfl)
            b2_sb = load(b2row[:], [1, H], bfl)
            bh1_sb = load(bh1v[:], [HH, 1], f32)
            bh2_sb = load(bh2v[:], [HH // 2, 1], f32)
            bh3_sb = load(bh3v[:], [1, 1], f32)
            gidx_sb = load(gidx[:], [P, gcols], i16)
            dsel_sb = load(dsel[:], [P, T * C], bfl)
            dinv_sb = load(dinv_tbl[:], [P, cfg.MT], f32)
            sqrow_sb = load(sqrow[:], [1, T * P], bfl)
            vidx_sb = load(vidx[:], [P, cfg.BCH], i32)
            ohT_sb = load(ohT[:], [OH, cfg.BPC], bfl)

            npad = cfg.SHP - cfg.T * P
            if npad > 0:
                zpad = const.tile([P, H], bfl)
                nc.any.memset(zpad[:], 0.0)
                nc.sync.dma_start(z1in[cfg.T * P:cfg.SHP, :], zpad[:npad, :])
                nc.sync.dma_start(h2in[cfg.T * P:cfg.SHP, :], zpad[:npad, :])

            # ---------------- phase A: conv1 z0 = dinv * (x @ Wc1)
            MBS = 7
            with tc.tile_pool(name="c1sb", bufs=3) as c1sb, \
                 tc.tile_pool(name="c1ev", bufs=3) as c1ev, \
                 tc.tile_pool(name="c1ps", bufs=MBS + 1, space="PSUM") as c1ps:
                for mb0 in range(0, cfg.MT, MBS):
                    mbn = min(MBS, cfg.MT - mb0)
                    accs = [c1ps.tile([P, H], f32, tag="convacc",
                                      name=f"convacc_{mb0}_{j}")
                            for j in range(mbn)]
                    for kt in range(cfg.KT):
                        slab = c1sb.tile([P, MBS * P], bfl, tag="slab")
                        nc.sync.dma_start(
                            slab[:, :mbn * P],
                            xT[kt * P:(kt + 1) * P, mb0 * P:(mb0 + mbn) * P])
                        for j in range(mbn):
                            nc.tensor.matmul(
                                accs[j][:], lhsT=slab[:, j * P:(j + 1) * P],
                                rhs=wc1_sb[:, kt, :],
                                start=(kt == 0), stop=(kt == cfg.KT - 1))
                    for j in range(mbn):
                        zb = c1ev.tile([P, H], bfl, tag="zev")
                        col = mb0 + j
                        nc.vector.tensor_scalar(
                            out=zb[:], in0=accs[j][:],
                            scalar1=dinv_sb[:, col:col + 1], scalar2=None,
                            op0=mybir.AluOpType.mult)
                        r0 = col * P
                        nc.sync.dma_start(z0in[r0:r0 + P, :], zb[:])

            nc.gpsimd.collective_compute(
                "AllGather", mybir.AluOpType.bypass, replica_groups=rg,
                ins=[z0in[:]], outs=[Z0[:]])

            # ---------------- aggregation layers
            def agg_layer(Z, locin, b_sb, out_dram, do_conv2, lname):
                with tc.tile_pool(name=f"agsb{lname}", bufs=4) as agsb, \
                     tc.tile_pool(name=f"agst{lname}", bufs=3) as agst, \
                     tc.tile_pool(name=f"agev{lname}", bufs=3) as agev, \
                     tc.tile_pool(name=f"agps{lname}", bufs=2,
                                  space="PSUM") as agps, \
                     tc.tile_pool(name=f"agp2{lname}", bufs=2,
                                  space="PSUM") as agp2:
                    Zlo = Z[:cfg.NPH, :]
                    Zhi = Z[cfg.NPH:, :]
                    qn = 0
                    for t in range(T):
                        ct = 1 + clo[t] + chi[t]
                        msg = agsb.tile([P, C, H], bfl, tag="msg")
                        # self chunk: contiguous local table rows
                        nc.sync.dma_start(msg[:, 0, :],
                                          locin[t * P:(t + 1) * P, :])
                        if nlo16[t] % P != 0 or nlo16[t] == 0:
                            nc.vector.memset(msg[:, 1 + nlo16[t] // P, :], 0.0)
                        if nlo16[t] > 0:
                            nc.gpsimd.dma_gather(
                                msg[:, 1:1 + clo[t], :], Zlo,
                                gidx_sb[:, lo_off_c[t]:lo_off_c[t + 1]],
                                nlo16[t], nlo16[t], H, single_packet=False,
                                queue_num=qn % NQ)
                            qn += 1
                        if nhi16[t] % P != 0 or nhi16[t] == 0:
                            nc.vector.memset(
                                msg[:, 1 + clo[t] + nhi16[t] // P, :], 0.0)
                        if nhi16[t] > 0:
                            nc.gpsimd.dma_gather(
                                msg[:, 1 + clo[t]:1 + clo[t] + chi[t], :], Zhi,
                                gidx_sb[:, hi_base + hi_off_c[t]:
                                        hi_base + hi_off_c[t + 1]],
                                nhi16[t], nhi16[t], H, single_packet=False,
                                queue_num=qn % NQ)
                            qn += 1
                        # one-hot build: st[p, c, j] = (dsel[p, c] == j)
                        st = agst.tile([P, C * P], bfl, tag="st")
                        dse = dsel_sb[:, t * C:t * C + ct]
                        dse_b = bass.AP(dse.tensor, dse.offset,
                                        [dse.ap[0], dse.ap[1], [0, P]])
                        io = iota_b[:]
                        io_b = bass.AP(io.tensor, io.offset,
                                       [io.ap[0], [0, ct], io.ap[1]])
                        ob = st[:, :ct * P].rearrange("p (c j) -> p c j", j=P)
                        nc.vector.tensor_tensor(out=ob, in0=io_b, in1=dse_b,
                                                op=mybir.AluOpType.is_equal)
                        acc = agps.tile([P, H], f32, tag="agacc")
                        nc.tensor.matmul(acc[:],
                                         lhsT=sqrow_sb[:, t * P:(t + 1) * P],
                                         rhs=b_sb[:], start=True, stop=False)
                        for ci in range(ct):
                            nc.tensor.matmul(acc[:],
                                             lhsT=st[:, ci * P:(ci + 1) * P],
                                             rhs=msg[:, ci, :],
                                             start=False,
                                             stop=(ci == ct - 1))
                        # h = relu(acc * dinv_d)
                        hb = agev.tile([P, H], bfl, tag="hb")
                        nc.vector.tensor_scalar(
                            out=hb[:], in0=acc[:],
                            scalar1=dinv_sb[:, t:t + 1], scalar2=0.0,
                            op0=mybir.AluOpType.mult,
                            op1=mybir.AluOpType.max)
                        if do_conv2:
                            ht = agev.tile([P, H], bfl, tag="ht")
                            for k in range(2):
                                pt = agp2.tile([P, P], bfl, space="PSUM",
                                               tag="pt")
                                nc.tensor.transpose(
                                    pt[:], hb[:, k * P:(k + 1) * P], ident[:])
                                nc.vector.tensor_copy(
                                    ht[:, k * P:(k + 1) * P], pt[:])
                            pz = agp2.tile([P, H], f32, tag="pz")
                            for k in range(2):
                                nc.tensor.matmul(
                                    pz[:], lhsT=ht[:, k * P:(k + 1) * P],
                                    rhs=wc2_sb[:, k, :],
                                    start=(k == 0), stop=(k == 1))
                            res = agev.tile([P, H], bfl, tag="res")
                            nc.vector.tensor_scalar(
                                out=res[:], in0=pz[:],
                                scalar1=dinv_sb[:, t:t + 1], scalar2=None,
                                op0=mybir.AluOpType.mult)
                        else:
                            res = hb
                        nc.sync.dma_start(out_dram[t * P:(t + 1) * P, :],
                                          res[:])

            lo_off_c = [0]
            for t in range(T):
                lo_off_c.append(lo_off_c[-1] + nlo16[t] // 16)
            hi_off_c = [0]
            for t in range(T):
                hi_off_c.append(hi_off_c[-1] + nhi16[t] // 16)

            agg_layer(Z0, z0in, b1_sb, z1in, do_conv2=True, lname="a")
            nc.gpsimd.collective_compute(
                "AllGather", mybir.AluOpType.bypass, replica_groups=rg,
                ins=[z1in[:]], outs=[Z1[:]])
            agg_layer(Z1, z1in, b2_sb, h2in, do_conv2=False, lname="b")
            nc.gpsimd.collective_compute(
                "AllGather", mybir.AluOpType.bypass, replica_groups=rg,
                ins=[h2in[:]], outs=[H2[:]])

            # ---------------- head
            with tc.tile_pool(name="hdsb", bufs=2) as hdsb, \
                 tc.tile_pool(name="hdps", bufs=2, space="PSUM") as hdps:
                zt0 = hdsb.tile([P, cfg.BPC], bfl, tag="zt0")
                zt1 = hdsb.tile([P, cfg.BPC], bfl, tag="zt1")
                for j in range(cfg.BCH):
                    g = hdsb.tile([P, H], bfl, tag="hg")
                    nc.gpsimd.indirect_dma_start(
                        out=g[:], out_offset=None, in_=H2[:],
                        in_offset=bass.IndirectOffsetOnAxis(
                            ap=vidx_sb[:, j:j + 1], axis=0))
                    for k in range(2):
                        pt = hdps.tile([P, P], bfl, space="PSUM", tag="hpt")
                        nc.tensor.transpose(pt[:], g[:, k * P:(k + 1) * P],
                                            ident[:])
                        dstt = zt0 if k == 0 else zt1
                        nc.vector.tensor_copy(
                            dstt[:, j * P:(j + 1) * P], pt[:])
                ph1 = hdps.tile([P, cfg.BPC], f32, tag="ph1")
                nc.tensor.matmul(ph1[:], lhsT=wh1_sb[:, 0, :], rhs=zt0[:],
                                 start=True, stop=False)
                nc.tensor.matmul(ph1[:], lhsT=wh1_sb[:, 1, :], rhs=zt1[:],
                                 start=False, stop=False)
                nc.tensor.matmul(ph1[:], lhsT=wh1_sb[:OH, 2, :],
                                 rhs=ohT_sb[:], start=False, stop=True)
                a1 = hdsb.tile([P, cfg.BPC], bfl, tag="a1")
                nc.scalar.activation(a1[:], ph1[:],
                                     mybir.ActivationFunctionType.Relu,
                                     bias=bh1_sb[:])
                ph2 = hdps.tile([HH // 2, cfg.BPC], f32, tag="ph2")
                nc.tensor.matmul(ph2[:], lhsT=wh2_sb[:], rhs=a1[:],
                                 start=True, stop=True)
                a2 = hdsb.tile([HH // 2, cfg.BPC], bfl, tag="a2")
                nc.scalar.activation(a2[:], ph2[:],
                                     mybir.ActivationFunctionType.Relu,
                                     bias=bh2_sb[:])
                ph3 = hdps.tile([1, cfg.BPC], f32, tag="ph3")
                nc.tensor.matmul(ph3[:], lhsT=wh3_sb[:], rhs=a2[:],
                                 start=True, stop=True)
                osb = hdsb.tile([1, cfg.BPC], f32, tag="osb")
                nc.vector.tensor_scalar_add(osb[:], ph3[:], bh3_sb[:, :1])
                nc.sync.dma_start(out[:], osb[:])

    nc.compile()
    return nc


# ------------------------------------------------------------------ driver

_CACHE = {}


def _get_program(cfg, meta):
    key = (cfg.N, cfg.E, cfg.D_IN, cfg.B, meta["CL"], meta["CH"],
           meta["nlo16"], meta["nhi16"])
    if key not in _CACHE:
        _CACHE[key] = build_program(cfg, meta)
    return _CACHE[key]


def kernel(**inputs):
    cfg = REAL
    in_maps, meta = host_prep(cfg, **inputs)
    nc = _get_program(cfg, meta)
    from concourse import bass_utils
    res = bass_utils.run_bass_kernel_spmd(
        nc, in_maps, core_ids=list(range(NCORES)))
    outs = [np.asarray(res.results[q]["out"]).reshape(cfg.BPC)
            for q in range(NCORES)]
    return np.concatenate(outs).astype(np.float32)


# revision 4
# speedup vs baseline: 1.4983x; 1.2218x over previous
"""Trainium2 Bass kernel for a 2-layer GCN encoder + MLP head (PyG GCNConv).

v2 strategy (8 NeuronCores, node-parallel):
  - Nodes sharded by contiguous range; dst tiles are contiguous 128-node
    blocks (T = ceil(SH/128)), so aggregation output writes are plain
    contiguous DMA (no indirect scatter).
  - norm = dinv[src]*dinv[dst] is factored: tables store dinv[i]*z[i]
    (scaled at conv evacuation); aggregation post-multiplies by dinv[d]
    fused with relu on DVE; bias enters via a 1-row matmul (sqdeg x b).
  - Self-loop edges form chunk 0 of each tile, loaded contiguously from the
    core-local table shard (z0in/z1in) with a plain dma_start.
  - Remaining edges: per tile two dma_gathers (lo/hi int16 index halves)
    with per-tile real counts (16-padded) and round-robin SWDGE queues
    (4 queues) to avoid descriptor-ring serialization.
  - One-hot routing matrices built on DVE with a single broadcast-AP
    is_equal per tile; aggregation = per-chunk one-hot matmuls in PSUM.
  - conv2 fused in the layer-1 tile loop via PE transposes.
All heavy compute bf16 with fp32 PSUM accumulation.
"""
import sys

for _p in ("/opt/trn_rl_repo",):
    if _p not in sys.path:
        sys.path.insert(0, _p)

import numpy as np
import ml_dtypes

bf16 = ml_dtypes.bfloat16

P = 128
H = 256          # gcn hidden width (fixed)
HH = 128         # head hidden width (fixed)
OH = 40          # wt_onehot + mut_onehot width (fixed)
NCORES = 8
NQ = 4           # SWDGE queues


class Cfg:
    def __init__(self, N, E, D_IN, B):
        self.N, self.E, self.D_IN, self.B = N, E, D_IN, B
        assert N % NCORES == 0
        self.SH = N // NCORES                      # real rows per shard
        self.T = -(-self.SH // P)                  # dst tiles per shard
        shp = max(self.T * P, -(-self.SH // P) * P)
        if shp == self.SH:
            shp += P
        self.SHP = -(-shp // P) * P                # padded rows per shard
        self.NP = NCORES * self.SHP                # padded global rows
        self.HSH = ((self.SHP // P + 1) // 2) * P  # A-half rows (P-aligned)
        self.HSB = self.SHP - self.HSH             # B-half rows
        self.TBL = NCORES * self.HSH               # A table rows
        self.TBB = NCORES * self.HSB               # B table rows
        assert self.TBL < 32768 and self.TBB < 32768
        self.KT = -(-D_IN // P)                    # k tiles for conv1
        self.KPAD = self.KT * P
        self.MT = self.SHP // P                    # m tiles per shard
        self.BPC = self.B // NCORES                # batch per core
        assert self.BPC % P == 0
        self.BCH = self.BPC // P                   # batch chunks


REAL = Cfg(N=50000, E=800000, D_IN=1281, B=4096)


# ---------------------------------------------------------------- host prep

def _pack_idx16(seq):
    """idx sequence [n] (n%16==0) -> wrapped-16 replicated [128, n//16] i16."""
    n = seq.shape[0]
    assert n % 16 == 0
    a = seq.reshape(n // 16, 16).T.astype(np.int16)
    return np.tile(a, (8, 1))


def host_prep(cfg, x, wt_onehot, mut_onehot, Wc1, bc1, Wc2, bc2,
              Wh1, bh1, Wh2, bh2, Wh3, bh3, edge_index, var_node_idx):
    N, E, SH, SHP, T = cfg.N, cfg.E, cfg.SH, cfg.SHP, cfg.T
    src = np.asarray(edge_index[0], np.int64)
    dst = np.asarray(edge_index[1], np.int64)
    # degree includes self loops (dst counts + 1)
    deg = (np.bincount(dst, minlength=N) + 1).astype(np.float32)
    dinv = (1.0 / np.sqrt(deg)).astype(np.float32)
    sqdeg = np.sqrt(deg).astype(np.float32)
    q_of = src // SH
    r_of = src % SH
    in_b = r_of >= cfg.HSH
    srcp = np.where(in_b, q_of * cfg.HSB + (r_of - cfg.HSH),
                    q_of * cfg.HSH + r_of)

    # drop self-referencing edges? (none in random data, but (i,i) entries in
    # edge_index are real edges distinct from the implicit self loop)
    core_of = dst // SH

    # per-core, per-tile edge lists
    per_core = []
    CLmax = CHmax = 0
    for q in range(NCORES):
        m = core_of == q
        d_loc = dst[m] - q * SH
        sp = srcp[m]
        tile_of = d_loc // P
        order = np.argsort(tile_of, kind="stable")
        d_s, sp_s, t_s = d_loc[order], sp[order], tile_of[order]
        tstarts = np.searchsorted(t_s, np.arange(T + 1))
        tiles = []
        inb_s = in_b[m][order]
        for t in range(T):
            a, b = tstarts[t], tstarts[t + 1]
            spt, dt_, ib = sp_s[a:b], d_s[a:b] - t * P, inb_s[a:b]
            lo = ~ib
            tiles.append(((spt[lo], dt_[lo]), (spt[ib], dt_[ib])))
            CLmax = max(CLmax, -(-int(lo.sum()) // P))
            CHmax = max(CHmax, -(-int(ib.sum()) // P))
        per_core.append(tiles)
    CL, CH = max(1, int(CLmax)), max(1, int(CHmax))
    C = 1 + CL + CH   # self chunk + lo chunks + hi chunks

    # per-tile static counts must be IDENTICAL across cores (one program):
    # use the max over cores for each tile's lo/hi counts.
    nlo = np.zeros(T, np.int64)
    nhi = np.zeros(T, np.int64)
    for q in range(NCORES):
        for t in range(T):
            (sl, _), (sh_, _) = per_core[q][t]
            nlo[t] = max(nlo[t], len(sl))
            nhi[t] = max(nhi[t], len(sh_))
    nlo16 = ((nlo + 15) // 16) * 16
    nhi16 = ((nhi + 15) // 16) * 16
    clo = np.maximum(1, -(-nlo16 // P))  # chunks per tile (>=1 for layout)
    chi = np.maximum(1, -(-nhi16 // P))
    # column offsets into gidx (units of 16-idx columns)
    lo_off = np.zeros(T + 1, np.int64)
    hi_off = np.zeros(T + 1, np.int64)
    np.cumsum(nlo16 // 16, out=lo_off[1:])
    hi_base = lo_off[T]
    np.cumsum(nhi16 // 16, out=hi_off[1:])
    gcols = int(lo_off[T] + hi_off[T])

    meta = dict(CL=CL, CH=CH, C=C,
                nlo16=tuple(int(v) for v in nlo16),
                nhi16=tuple(int(v) for v in nhi16),
                clo=tuple(int(v) for v in clo),
                chi=tuple(int(v) for v in chi),
                gcols=gcols, hi_base=int(hi_base))

    # shared weights
    wc1 = np.zeros((cfg.KPAD, H), bf16)
    wc1[:cfg.D_IN] = np.asarray(Wc1, np.float32).astype(bf16)
    wc2 = np.asarray(Wc2, np.float32).astype(bf16)
    wh1 = np.zeros((3 * P, HH), bf16)
    wh1[:H + OH] = np.asarray(Wh1, np.float32).astype(bf16)
    wh2 = np.asarray(Wh2, np.float32).astype(bf16)
    wh3 = np.asarray(Wh3, np.float32).astype(bf16)
    b1row = np.asarray(bc1, np.float32).astype(bf16).reshape(1, H)
    b2row = np.asarray(bc2, np.float32).astype(bf16).reshape(1, H)
    bh1v = np.asarray(bh1, np.float32).reshape(HH, 1)
    bh2v = np.asarray(bh2, np.float32).reshape(HH // 2, 1)
    bh3v = np.asarray(bh3, np.float32).reshape(1, 1)

    x = np.asarray(x, np.float32)
    wt_b = np.asarray(wt_onehot, np.float32).astype(bf16)
    mut_b = np.asarray(mut_onehot, np.float32).astype(bf16)
    vni = np.asarray(var_node_idx, np.int64)
    vq, vr = vni // SH, vni % SH
    vrow = np.where(vr >= cfg.HSH,
                    cfg.TBL + vq * cfg.HSB + (vr - cfg.HSH),
                    vq * cfg.HSH + vr)

    in_maps = []
    for q in range(NCORES):
        gidx_seq = np.zeros(gcols * 16, np.int64)
        dsel = np.full((P, T * C), 999.0, np.float32)
        for t in range(T):
            (sl, dl), (sh_, dh_) = per_core[q][t]
            # self chunk col: diagonal for real rows
            nreal = min(P, SH - t * P)
            dsel[:nreal, t * C] = np.arange(nreal, dtype=np.float32)
            base = lo_off[t] * 16
            gidx_seq[base:base + len(sl)] = sl
            a_d = np.full(clo[t] * P, 999.0, np.float32)
            a_d[:len(dl)] = dl
            dsel[:, t * C + 1:t * C + 1 + clo[t]] = \
                a_d.reshape(clo[t], P).T
            base = (hi_base + hi_off[t]) * 16
            gidx_seq[base:base + len(sh_)] = sh_
            a_d = np.full(chi[t] * P, 999.0, np.float32)
            a_d[:len(dh_)] = dh_
            dsel[:, t * C + 1 + clo[t]:t * C + 1 + clo[t] + chi[t]] = \
                a_d.reshape(chi[t], P).T
        gidx = _pack_idx16(gidx_seq)

        # dinv per m-tile column [128, MT]; sqdeg row [1, T*P]
        dloc = np.zeros(SHP, np.float32)
        dloc[:SH] = dinv[q * SH:(q + 1) * SH]
        dinv_tbl = dloc.reshape(cfg.MT, P).T.copy()
        sq = np.zeros(T * P, np.float32)
        sq[:SH] = sqdeg[q * SH:(q + 1) * SH]
        sqrow = sq.reshape(1, T * P).astype(bf16)

        xT = np.zeros((cfg.KPAD, SHP), bf16)
        xT[:cfg.D_IN, :SH] = x[q * SH:(q + 1) * SH].T.astype(bf16)
        vr = vrow[q * cfg.BPC:(q + 1) * cfg.BPC]
        vidx = vr.reshape(cfg.BCH, P).T.astype(np.int32)
        ohT = np.concatenate(
            [wt_b[q * cfg.BPC:(q + 1) * cfg.BPC].T,
             mut_b[q * cfg.BPC:(q + 1) * cfg.BPC].T], axis=0)  # [40, BPC]
        in_maps.append(dict(
            xT=xT, gidx=gidx, dsel=dsel.astype(bf16),
            dinv_tbl=np.ascontiguousarray(dinv_tbl),
            sqrow=np.ascontiguousarray(sqrow),
            vidx=np.ascontiguousarray(vidx), ohT=np.ascontiguousarray(ohT),
            wc1=wc1, wc2=wc2, wh1=wh1, wh2=wh2, wh3=wh3,
            b1row=b1row, b2row=b2row,
            bh1v=bh1v, bh2v=bh2v, bh3v=bh3v,
        ))
    return in_maps, meta


# ------------------------------------------------------------- bass program

def build_program(cfg, meta):
    import concourse.bass as bass
    import concourse.mybir as mybir
    import concourse.tile as tile
    from concourse import bacc
    from concourse.masks import make_identity

    T = cfg.T
    CL, CH, C = meta["CL"], meta["CH"], meta["C"]
    nlo16, nhi16 = meta["nlo16"], meta["nhi16"]
    clo, chi = meta["clo"], meta["chi"]
    gcols, hi_base = meta["gcols"], meta["hi_base"]

    nc = bacc.Bacc("TRN2", target_bir_lowering=False, debug=False,
                   num_devices=NCORES, num_swdge_queues=NQ)
    f32, bfl, i16, i32 = (mybir.dt.float32, mybir.dt.bfloat16,
                          mybir.dt.int16, mybir.dt.int32)
    fp8 = mybir.dt.float8e4

    # I/O
    xT = nc.dram_tensor("xT", [cfg.KPAD, cfg.SHP], bfl, kind="ExternalInput")
    gidx = nc.dram_tensor("gidx", [P, gcols], i16, kind="ExternalInput")
    dsel = nc.dram_tensor("dsel", [P, T * C], bfl, kind="ExternalInput")
    dinv_tbl = nc.dram_tensor("dinv_tbl", [P, cfg.MT], f32,
                              kind="ExternalInput")
    sqrow = nc.dram_tensor("sqrow", [1, T * P], bfl, kind="ExternalInput")
    vidx = nc.dram_tensor("vidx", [P, cfg.BCH], i32, kind="ExternalInput")
    ohT = nc.dram_tensor("ohT", [OH, cfg.BPC], bfl, kind="ExternalInput")
    wc1 = nc.dram_tensor("wc1", [cfg.KPAD, H], bfl, kind="ExternalInput")
    wc2 = nc.dram_tensor("wc2", [H, H], bfl, kind="ExternalInput")
    wh1 = nc.dram_tensor("wh1", [3 * P, HH], bfl, kind="ExternalInput")
    wh2 = nc.dram_tensor("wh2", [HH, HH // 2], bfl, kind="ExternalInput")
    wh3 = nc.dram_tensor("wh3", [HH // 2, 1], bfl, kind="ExternalInput")
    b1row = nc.dram_tensor("b1row", [1, H], bfl, kind="ExternalInput")
    b2row = nc.dram_tensor("b2row", [1, H], bfl, kind="ExternalInput")
    bh1v = nc.dram_tensor("bh1v", [HH, 1], f32, kind="ExternalInput")
    bh2v = nc.dram_tensor("bh2v", [HH // 2, 1], f32, kind="ExternalInput")
    bh3v = nc.dram_tensor("bh3v", [1, 1], f32, kind="ExternalInput")
    out = nc.dram_tensor("out", [1, cfg.BPC], f32, kind="ExternalOutput")

    # internal DRAM
    z0in = nc.dram_tensor("z0in", [cfg.SHP, H], fp8, kind="Internal")
    z1in = nc.dram_tensor("z1in", [cfg.SHP, H], fp8, kind="Internal")
    h2in = nc.dram_tensor("h2in", [cfg.SHP, H], bfl, kind="Internal")
    Z0a = nc.dram_tensor("Z0a", [cfg.TBL, H], fp8, kind="Internal",
                         addr_space="Shared")
    Z0b = nc.dram_tensor("Z0b", [cfg.TBB, H], fp8, kind="Internal",
                         addr_space="Shared")
    Z1a = nc.dram_tensor("Z1a", [cfg.TBL, H], fp8, kind="Internal",
                         addr_space="Shared")
    Z1b = nc.dram_tensor("Z1b", [cfg.TBB, H], fp8, kind="Internal",
                         addr_space="Shared")
    H2 = nc.dram_tensor("H2", [cfg.TBL + cfg.TBB, H], bfl, kind="Internal",
                        addr_space="Shared")
    rg = [list(range(NCORES))]

    with tile.TileContext(nc) as tc:
        with tc.tile_pool(name="const", bufs=1) as const:
            iota_i = const.tile([P, P], i32)
            nc.gpsimd.iota(iota_i[:], pattern=[[1, P]], base=0,
                           channel_multiplier=0)
            iota_b = const.tile([P, P], bfl)
            nc.vector.tensor_copy(iota_b[:], iota_i[:])
            ident = const.tile([P, P], bfl)
            make_identity(nc, ident[:])

            def load(ap, shape, dt):
                t = const.tile(shape, dt, tag=ap.tensor.name)
                nc.sync.dma_start(t[:], ap)
                return t

            wc1_sb = load(wc1.rearrange("(t p) n -> p t n", p=P)[:],
                          [P, cfg.KT, H], bfl)
            wc2_sb = load(wc2.rearrange("(t p) n -> p t n", p=P)[:],
                          [P, 2, H], bfl)
            wh1_sb = load(wh1.rearrange("(t p) n -> p t n", p=P)[:],
                          [P, 3, HH], bfl)
            wh2_sb = load(wh2[:], [HH, HH // 2], bfl)
            wh3_sb = load(wh3[:], [HH // 2, 1], bfl)
            b1_sb = load(b1row[:], [1, H], bfl)
            b2_sb = load(b2row[:], [1, H], bfl)
            bh1_sb = load(bh1v[:], [HH, 1], f32)
            bh2_sb = load(bh2v[:], [HH // 2, 1], f32)
            bh3_sb = load(bh3v[:], [1, 1], f32)
            gidx_sb = load(gidx[:], [P, gcols], i16)
            dsel_sb = load(dsel[:], [P, T * C], bfl)
            dinv_sb = load(dinv_tbl[:], [P, cfg.MT], f32)
            sqrow_sb = load(sqrow[:], [1, T * P], bfl)
            vidx_sb = load(vidx[:], [P, cfg.BCH], i32)
            ohT_sb = load(ohT[:], [OH, cfg.BPC], bfl)

            npad = cfg.SHP - cfg.T * P
            if npad > 0:
                zpad = const.tile([P, H], bfl)
                nc.any.memset(zpad[:], 0.0)
                zpad8 = const.tile([P, H], fp8)
                nc.any.memset(zpad8[:], 0.0)
                nc.sync.dma_start(z1in[cfg.T * P:cfg.SHP, :], zpad8[:npad, :])
                nc.sync.dma_start(h2in[cfg.T * P:cfg.SHP, :], zpad[:npad, :])

            # ---------------- phase A: conv1 z0 = dinv * (x @ Wc1)
            MBS = 7
            with tc.tile_pool(name="c1sb", bufs=3) as c1sb, \
                 tc.tile_pool(name="c1ev", bufs=3) as c1ev, \
                 tc.tile_pool(name="c1ps", bufs=MBS + 1, space="PSUM") as c1ps:
                for mb0 in range(0, cfg.MT, MBS):
                    mbn = min(MBS, cfg.MT - mb0)
                    accs = [c1ps.tile([P, H], f32, tag="convacc",
                                      name=f"convacc_{mb0}_{j}")
                            for j in range(mbn)]
                    for kt in range(cfg.KT):
                        slab = c1sb.tile([P, MBS * P], bfl, tag="slab")
                        nc.sync.dma_start(
                            slab[:, :mbn * P],
                            xT[kt * P:(kt + 1) * P, mb0 * P:(mb0 + mbn) * P])
                        for j in range(mbn):
                            nc.tensor.matmul(
                                accs[j][:], lhsT=slab[:, j * P:(j + 1) * P],
                                rhs=wc1_sb[:, kt, :],
                                start=(kt == 0), stop=(kt == cfg.KT - 1))
                    for j in range(mbn):
                        zb = c1ev.tile([P, H], fp8, tag="zev")
                        col = mb0 + j
                        nc.vector.tensor_scalar(
                            out=zb[:], in0=accs[j][:],
                            scalar1=dinv_sb[:, col:col + 1], scalar2=None,
                            op0=mybir.AluOpType.mult)
                        r0 = col * P
                        nc.sync.dma_start(z0in[r0:r0 + P, :], zb[:])
                    if mb0 + mbn >= cfg.HSH // P and mb0 < cfg.HSH // P:
                        nc.gpsimd.collective_compute(
                            "AllGather", mybir.AluOpType.bypass,
                            replica_groups=rg,
                            ins=[z0in[:cfg.HSH, :]], outs=[Z0a[:]])

            nc.gpsimd.collective_compute(
                "AllGather", mybir.AluOpType.bypass, replica_groups=rg,
                ins=[z0in[cfg.HSH:, :]], outs=[Z0b[:]])

            # ---------------- aggregation layers
            def agg_layer(Za, Zb, locin, b_sb, out_dram, do_conv2, lname,
                          mdt, odt, post_tile=None):
                with tc.tile_pool(name=f"agsb{lname}", bufs=6) as agsb, \
                     tc.tile_pool(name=f"agst{lname}", bufs=4) as agst, \
                     tc.tile_pool(name=f"agev{lname}", bufs=3) as agev, \
                     tc.tile_pool(name=f"agps{lname}", bufs=3,
                                  space="PSUM") as agps, \
                     tc.tile_pool(name=f"agp2{lname}", bufs=2,
                                  space="PSUM") as agp2:
                    qn = 0
                    for t in range(T):
                        ct = 1 + clo[t] + chi[t]
                        msg = agsb.tile([P, C, H], mdt, tag="msg")
                        # self chunk: contiguous local table rows
                        nc.sync.dma_start(msg[:, 0, :],
                                          locin[t * P:(t + 1) * P, :])
                        if nlo16[t] % P != 0 or nlo16[t] == 0:
                            nc.vector.memset(msg[:, 1 + nlo16[t] // P, :], 0.0)
                        if nlo16[t] > 0:
                            nc.gpsimd.dma_gather(
                                msg[:, 1:1 + clo[t], :], Za[:],
                                gidx_sb[:, lo_off_c[t]:lo_off_c[t + 1]],
                                nlo16[t], nlo16[t], H, single_packet=False,
                                queue_num=qn % NQ)
                            qn += 1
                        if nhi16[t] % P != 0 or nhi16[t] == 0:
                            nc.vector.memset(
                                msg[:, 1 + clo[t] + nhi16[t] // P, :], 0.0)
                        if nhi16[t] > 0:
                            nc.gpsimd.dma_gather(
                                msg[:, 1 + clo[t]:1 + clo[t] + chi[t], :],
                                Zb[:],
                                gidx_sb[:, hi_base + hi_off_c[t]:
                                        hi_base + hi_off_c[t + 1]],
                                nhi16[t], nhi16[t], H, single_packet=False,
                                queue_num=qn % NQ)
                            qn += 1
                        # one-hot build: st[p, c, j] = (dsel[p, c] == j)
                        st = agst.tile([P, C * P], mdt, tag="st")
                        dse = dsel_sb[:, t * C:t * C + ct]
                        dse_b = bass.AP(dse.tensor, dse.offset,
                                        [dse.ap[0], dse.ap[1], [0, P]])
                        io = iota_b[:]
                        io_b = bass.AP(io.tensor, io.offset,
                                       [io.ap[0], [0, ct], io.ap[1]])
                        ob = st[:, :ct * P].rearrange("p (c j) -> p c j", j=P)
                        nc.vector.tensor_tensor(out=ob, in0=io_b, in1=dse_b,
                                                op=mybir.AluOpType.is_equal)
                        acc = agps.tile([P, H], f32, tag="agacc")
                        nc.tensor.matmul(acc[:],
                                         lhsT=sqrow_sb[:, t * P:(t + 1) * P],
                                         rhs=b_sb[:], start=True, stop=False)
                        for ci in range(ct):
                            nc.tensor.matmul(acc[:],
                                             lhsT=st[:, ci * P:(ci + 1) * P],
                                             rhs=msg[:, ci, :],
                                             start=False,
                                             stop=(ci == ct - 1))
                        # h = relu(acc * dinv_d)
                        hb = agev.tile([P, H], bfl if do_conv2 else odt,
                                       tag="hb")
                        nc.vector.tensor_scalar(
                            out=hb[:], in0=acc[:],
                            scalar1=dinv_sb[:, t:t + 1], scalar2=0.0,
                            op0=mybir.AluOpType.mult,
                            op1=mybir.AluOpType.max)
                        if do_conv2:
                            ht = agev.tile([P, H], bfl, tag="ht")
                            for k in range(2):
                                pt = agp2.tile([P, P], bfl, space="PSUM",
                                               tag="pt")
                                nc.tensor.transpose(
                                    pt[:], hb[:, k * P:(k + 1) * P], ident[:])
                                nc.vector.tensor_copy(
                                    ht[:, k * P:(k + 1) * P], pt[:])
                            pz = agp2.tile([P, H], f32, tag="pz")
                            for k in range(2):
                                nc.tensor.matmul(
                                    pz[:], lhsT=ht[:, k * P:(k + 1) * P],
                                    rhs=wc2_sb[:, k, :],
                                    start=(k == 0), stop=(k == 1))
                            res = agev.tile([P, H], odt, tag="res")
                            nc.vector.tensor_scalar(
                                out=res[:], in0=pz[:],
                                scalar1=dinv_sb[:, t:t + 1], scalar2=None,
                                op0=mybir.AluOpType.mult)
                        else:
                            res = hb
                        nc.sync.dma_start(out_dram[t * P:(t + 1) * P, :],
                                          res[:])
                        if post_tile is not None:
                            post_tile(t)

            lo_off_c = [0]
            for t in range(T):
                lo_off_c.append(lo_off_c[-1] + nlo16[t] // 16)
            hi_off_c = [0]
            for t in range(T):
                hi_off_c.append(hi_off_c[-1] + nhi16[t] // 16)

            half_t = cfg.HSH // P - 1   # last tile of the A half

            def post1(t):
                if t == half_t:
                    nc.gpsimd.collective_compute(
                        "AllGather", mybir.AluOpType.bypass,
                        replica_groups=rg,
                        ins=[z1in[:cfg.HSH, :]], outs=[Z1a[:]])

            agg_layer(Z0a, Z0b, z0in, b1_sb, z1in, do_conv2=True, lname="a",
                      mdt=fp8, odt=fp8, post_tile=post1)
            nc.gpsimd.collective_compute(
                "AllGather", mybir.AluOpType.bypass, replica_groups=rg,
                ins=[z1in[cfg.HSH:, :]], outs=[Z1b[:]])

            def post2(t):
                if t == half_t:
                    nc.gpsimd.collective_compute(
                        "AllGather", mybir.AluOpType.bypass,
                        replica_groups=rg,
                        ins=[h2in[:cfg.HSH, :]], outs=[H2[:cfg.TBL, :]])

            agg_layer(Z1a, Z1b, z1in, b2_sb, h2in, do_conv2=False, lname="b",
                      mdt=fp8, odt=bfl, post_tile=post2)
            nc.gpsimd.collective_compute(
                "AllGather", mybir.AluOpType.bypass, replica_groups=rg,
                ins=[h2in[cfg.HSH:, :]], outs=[H2[cfg.TBL:, :]])

            # ---------------- head
            with tc.tile_pool(name="hdsb", bufs=2) as hdsb, \
                 tc.tile_pool(name="hdps", bufs=2, space="PSUM") as hdps:
                zt0 = hdsb.tile([P, cfg.BPC], bfl, tag="zt0")
                zt1 = hdsb.tile([P, cfg.BPC], bfl, tag="zt1")
                for j in range(cfg.BCH):
                    g = hdsb.tile([P, H], bfl, tag="hg")
                    nc.gpsimd.indirect_dma_start(
                        out=g[:], out_offset=None, in_=H2[:],
                        in_offset=bass.IndirectOffsetOnAxis(
                            ap=vidx_sb[:, j:j + 1], axis=0))
                    for k in range(2):
                        pt = hdps.tile([P, P], bfl, space="PSUM", tag="hpt")
                        nc.tensor.transpose(pt[:], g[:, k * P:(k + 1) * P],
                                            ident[:])
                        dstt = zt0 if k == 0 else zt1
                        nc.vector.tensor_copy(
                            dstt[:, j * P:(j + 1) * P], pt[:])
                ph1 = hdps.tile([P, cfg.BPC], f32, tag="ph1")
                nc.tensor.matmul(ph1[:], lhsT=wh1_sb[:, 0, :], rhs=zt0[:],
                                 start=True, stop=False)
                nc.tensor.matmul(ph1[:], lhsT=wh1_sb[:, 1, :], rhs=zt1[:],
                                 start=False, stop=False)
                nc.tensor.matmul(ph1[:], lhsT=wh1_sb[:OH, 2, :],
                                 rhs=ohT_sb[:], start=False, stop=True)
                a1 = hdsb.tile([P, cfg.BPC], bfl, tag="a1")
                nc.scalar.activation(a1[:], ph1[:],
                                     mybir.ActivationFunctionType.Relu,
                                     bias=bh1_sb[:])
                ph2 = hdps.tile([HH // 2, cfg.BPC], f32, tag="ph2")
                nc.tensor.matmul(ph2[:], lhsT=wh2_sb[:], rhs=a1[:],
                                 start=True, stop=True)
                a2 = hdsb.tile([HH // 2, cfg.BPC], bfl, tag="a2")
                nc.scalar.activation(a2[:], ph2[:],
                                     mybir.ActivationFunctionType.Relu,
                                     bias=bh2_sb[:])
                ph3 = hdps.tile([1, cfg.BPC], f32, tag="ph3")
                nc.tensor.matmul(ph3[:], lhsT=wh3_sb[:], rhs=a2[:],
                                 start=True, stop=True)
                osb = hdsb.tile([1, cfg.BPC], f32, tag="osb")
                nc.vector.tensor_scalar_add(osb[:], ph3[:], bh3_sb[:, :1])
                nc.sync.dma_start(out[:], osb[:])

    nc.compile()
    return nc


# ------------------------------------------------------------------ driver

_CACHE = {}


def _get_program(cfg, meta):
    key = (cfg.N, cfg.E, cfg.D_IN, cfg.B, meta["CL"], meta["CH"],
           meta["nlo16"], meta["nhi16"])
    if key not in _CACHE:
        _CACHE[key] = build_program(cfg, meta)
    return _CACHE[key]


def kernel(**inputs):
    cfg = REAL
    in_maps, meta = host_prep(cfg, **inputs)
    nc = _get_program(cfg, meta)
    from concourse import bass_utils
    res = bass_utils.run_bass_kernel_spmd(
        nc, in_maps, core_ids=list(range(NCORES)))
    outs = [np.asarray(res.results[q]["out"]).reshape(cfg.BPC)
            for q in range(NCORES)]
    return np.concatenate(outs).astype(np.float32)


# revision 5
# speedup vs baseline: 1.6561x; 1.1053x over previous
"""Trainium2 Bass kernel for a 2-layer GCN encoder + MLP head (PyG GCNConv).

v2 strategy (8 NeuronCores, node-parallel):
  - Nodes sharded by contiguous range; dst tiles are contiguous 128-node
    blocks (T = ceil(SH/128)), so aggregation output writes are plain
    contiguous DMA (no indirect scatter).
  - norm = dinv[src]*dinv[dst] is factored: tables store dinv[i]*z[i]
    (scaled at conv evacuation); aggregation post-multiplies by dinv[d]
    fused with relu on DVE; bias enters via a 1-row matmul (sqdeg x b).
  - Self-loop edges form chunk 0 of each tile, loaded contiguously from the
    core-local table shard (z0in/z1in) with a plain dma_start.
  - Remaining edges: per tile two dma_gathers (lo/hi int16 index halves)
    with per-tile real counts (16-padded) and round-robin SWDGE queues
    (4 queues) to avoid descriptor-ring serialization.
  - One-hot routing matrices built on DVE with a single broadcast-AP
    is_equal per tile; aggregation = per-chunk one-hot matmuls in PSUM.
  - conv2 fused in the layer-1 tile loop via PE transposes.
All heavy compute bf16 with fp32 PSUM accumulation.
"""
import sys

for _p in ("/opt/trn_rl_repo",):
    if _p not in sys.path:
        sys.path.insert(0, _p)

import numpy as np
import ml_dtypes

bf16 = ml_dtypes.bfloat16

P = 128
H = 256          # gcn hidden width (fixed)
HH = 128         # head hidden width (fixed)
OH = 40          # wt_onehot + mut_onehot width (fixed)
NCORES = 8
NQ = 4           # SWDGE queues


class Cfg:
    def __init__(self, N, E, D_IN, B):
        self.N, self.E, self.D_IN, self.B = N, E, D_IN, B
        assert N % NCORES == 0
        self.SH = N // NCORES                      # real rows per shard
        self.T = -(-self.SH // P)                  # dst tiles per shard
        shp = max(self.T * P, -(-self.SH // P) * P)
        if shp == self.SH:
            shp += P
        self.SHP = -(-shp // P) * P                # padded rows per shard
        self.NP = NCORES * self.SHP                # padded global rows
        self.HSH = ((self.SHP // P + 1) // 2) * P  # A-half rows (P-aligned)
        self.HSB = self.SHP - self.HSH             # B-half rows
        self.TBL = NCORES * self.HSH               # A table rows
        self.TBB = NCORES * self.HSB               # B table rows
        assert self.TBL < 32768 and self.TBB < 32768
        self.KT = -(-D_IN // P)                    # k tiles for conv1
        self.KPAD = self.KT * P
        self.MT = self.SHP // P                    # m tiles per shard
        self.BPC = self.B // NCORES                # batch per core
        assert self.BPC % P == 0
        self.BCH = self.BPC // P                   # batch chunks


REAL = Cfg(N=50000, E=800000, D_IN=1281, B=4096)


# ---------------------------------------------------------------- host prep

def _pack_idx16(seq):
    """idx sequence [n] (n%16==0) -> wrapped-16 replicated [128, n//16] i16."""
    n = seq.shape[0]
    assert n % 16 == 0
    a = seq.reshape(n // 16, 16).T.astype(np.int16)
    return np.tile(a, (8, 1))


def host_prep(cfg, x, wt_onehot, mut_onehot, Wc1, bc1, Wc2, bc2,
              Wh1, bh1, Wh2, bh2, Wh3, bh3, edge_index, var_node_idx):
    N, E, SH, SHP, T = cfg.N, cfg.E, cfg.SH, cfg.SHP, cfg.T
    src = np.asarray(edge_index[0], np.int64)
    dst = np.asarray(edge_index[1], np.int64)
    # degree includes self loops (dst counts + 1)
    deg = (np.bincount(dst, minlength=N) + 1).astype(np.float32)
    dinv = (1.0 / np.sqrt(deg)).astype(np.float32)
    sqdeg = np.sqrt(deg).astype(np.float32)
    q_of = src // SH
    r_of = src % SH
    in_b = r_of >= cfg.HSH
    srcp = np.where(in_b, q_of * cfg.HSB + (r_of - cfg.HSH),
                    q_of * cfg.HSH + r_of)

    # drop self-referencing edges? (none in random data, but (i,i) entries in
    # edge_index are real edges distinct from the implicit self loop)
    core_of = dst // SH

    # per-core, per-tile edge lists
    per_core = []
    CLmax = CHmax = 0
    for q in range(NCORES):
        m = core_of == q
        d_loc = dst[m] - q * SH
        sp = srcp[m]
        tile_of = d_loc // P
        order = np.argsort(tile_of, kind="stable")
        d_s, sp_s, t_s = d_loc[order], sp[order], tile_of[order]
        tstarts = np.searchsorted(t_s, np.arange(T + 1))
        tiles = []
        inb_s = in_b[m][order]
        for t in range(T):
            a, b = tstarts[t], tstarts[t + 1]
            spt, dt_, ib = sp_s[a:b], d_s[a:b] - t * P, inb_s[a:b]
            lo = ~ib
            tiles.append(((spt[lo], dt_[lo]), (spt[ib], dt_[ib])))
            CLmax = max(CLmax, -(-int(lo.sum()) // P))
            CHmax = max(CHmax, -(-int(ib.sum()) // P))
        per_core.append(tiles)
    CL, CH = max(1, int(CLmax)), max(1, int(CHmax))
    C = 1 + CL + CH   # self chunk + lo chunks + hi chunks

    # per-tile static counts must be IDENTICAL across cores (one program):
    # use the max over cores for each tile's lo/hi counts.
    nlo = np.zeros(T, np.int64)
    nhi = np.zeros(T, np.int64)
    for q in range(NCORES):
        for t in range(T):
            (sl, _), (sh_, _) = per_core[q][t]
            nlo[t] = max(nlo[t], len(sl))
            nhi[t] = max(nhi[t], len(sh_))
    nlo16 = ((nlo + 15) // 16) * 16
    nhi16 = ((nhi + 15) // 16) * 16
    clo = np.maximum(1, -(-nlo16 // P))  # chunks per tile (>=1 for layout)
    chi = np.maximum(1, -(-nhi16 // P))
    # column offsets into gidx (units of 16-idx columns)
    lo_off = np.zeros(T + 1, np.int64)
    hi_off = np.zeros(T + 1, np.int64)
    np.cumsum(nlo16 // 16, out=lo_off[1:])
    hi_base = lo_off[T]
    np.cumsum(nhi16 // 16, out=hi_off[1:])
    gcols = int(lo_off[T] + hi_off[T])

    vni = np.asarray(var_node_idx, np.int64)
    vq, vr = vni // SH, vni % SH
    positions = [np.nonzero(vq == q)[0] for q in range(NCORES)]
    bmax = max(len(p) for p in positions)
    BMAX = ((bmax + P - 1) // P) * P

    meta = dict(CL=CL, CH=CH, C=C, BMAX=BMAX, positions=positions,
                nlo16=tuple(int(v) for v in nlo16),
                nhi16=tuple(int(v) for v in nhi16),
                clo=tuple(int(v) for v in clo),
                chi=tuple(int(v) for v in chi),
                gcols=gcols, hi_base=int(hi_base))

    # shared weights
    wc1 = np.zeros((cfg.KPAD, H), bf16)
    wc1[:cfg.D_IN] = np.asarray(Wc1, np.float32).astype(bf16)
    wc2 = np.asarray(Wc2, np.float32).astype(bf16)
    wh1 = np.zeros((3 * P, HH), bf16)
    wh1[:H + OH] = np.asarray(Wh1, np.float32).astype(bf16)
    wh2 = np.asarray(Wh2, np.float32).astype(bf16)
    wh3 = np.asarray(Wh3, np.float32).astype(bf16)
    b1row = np.asarray(bc1, np.float32).astype(bf16).reshape(1, H)
    b2row = np.asarray(bc2, np.float32).astype(bf16).reshape(1, H)
    bh1v = np.asarray(bh1, np.float32).reshape(HH, 1)
    bh2v = np.asarray(bh2, np.float32).reshape(HH // 2, 1)
    bh3v = np.asarray(bh3, np.float32).reshape(1, 1)

    x = np.asarray(x, np.float32)
    wt_b = np.asarray(wt_onehot, np.float32).astype(bf16)
    mut_b = np.asarray(mut_onehot, np.float32).astype(bf16)

    in_maps = []
    for q in range(NCORES):
        gidx_seq = np.zeros(gcols * 16, np.int64)
        dsel = np.full((P, T * C), 999.0, np.float32)
        for t in range(T):
            (sl, dl), (sh_, dh_) = per_core[q][t]
            # self chunk col: diagonal for real rows
            nreal = min(P, SH - t * P)
            dsel[:nreal, t * C] = np.arange(nreal, dtype=np.float32)
            base = lo_off[t] * 16
            gidx_seq[base:base + len(sl)] = sl
            a_d = np.full(clo[t] * P, 999.0, np.float32)
            a_d[:len(dl)] = dl
            dsel[:, t * C + 1:t * C + 1 + clo[t]] = \
                a_d.reshape(clo[t], P).T
            base = (hi_base + hi_off[t]) * 16
            gidx_seq[base:base + len(sh_)] = sh_
            a_d = np.full(chi[t] * P, 999.0, np.float32)
            a_d[:len(dh_)] = dh_
            dsel[:, t * C + 1 + clo[t]:t * C + 1 + clo[t] + chi[t]] = \
                a_d.reshape(chi[t], P).T
        gidx = _pack_idx16(gidx_seq)

        # dinv per m-tile column [128, MT]; sqdeg row [1, T*P]
        dloc = np.zeros(SHP, np.float32)
        dloc[:SH] = dinv[q * SH:(q + 1) * SH]
        dinv_tbl = dloc.reshape(cfg.MT, P).T.copy()
        sq = np.zeros(T * P, np.float32)
        sq[:SH] = sqdeg[q * SH:(q + 1) * SH]
        sqrow = sq.reshape(1, T * P).astype(bf16)

        xT = np.zeros((cfg.KPAD, SHP), bf16)
        xT[:cfg.D_IN, :SH] = x[q * SH:(q + 1) * SH].T.astype(bf16)
        pos = positions[q]
        vloc = np.zeros(BMAX, np.int64)
        vloc[:len(pos)] = vr[pos]
        vidx = vloc.reshape(BMAX // P, P).T.astype(np.int32)
        ohp = np.zeros((BMAX, 2 * 20), bf16)
        ohp[:len(pos), :20] = wt_b[pos]
        ohp[:len(pos), 20:] = mut_b[pos]
        ohT = ohp.T.copy()  # [40, BMAX]
        in_maps.append(dict(
            xT=xT, gidx=gidx, dsel=dsel.astype(bf16),
            dinv_tbl=np.ascontiguousarray(dinv_tbl),
            sqrow=np.ascontiguousarray(sqrow),
            vidx=np.ascontiguousarray(vidx), ohT=np.ascontiguousarray(ohT),
            wc1=wc1, wc2=wc2, wh1=wh1, wh2=wh2, wh3=wh3,
            b1row=b1row, b2row=b2row,
            bh1v=bh1v, bh2v=bh2v, bh3v=bh3v,
        ))
    return in_maps, meta


# ------------------------------------------------------------- bass program

def build_program(cfg, meta):
    import concourse.bass as bass
    import concourse.mybir as mybir
    import concourse.tile as tile
    from concourse import bacc
    from concourse.masks import make_identity

    T = cfg.T
    CL, CH, C = meta["CL"], meta["CH"], meta["C"]
    nlo16, nhi16 = meta["nlo16"], meta["nhi16"]
    clo, chi = meta["clo"], meta["chi"]
    gcols, hi_base = meta["gcols"], meta["hi_base"]
    BMAX = meta["BMAX"]
    BCH2 = BMAX // P

    nc = bacc.Bacc("TRN2", target_bir_lowering=False, debug=False,
                   num_devices=NCORES, num_swdge_queues=NQ,
                   dynamic_dma_scratch_size=49152)
    f32, bfl, i16, i32 = (mybir.dt.float32, mybir.dt.bfloat16,
                          mybir.dt.int16, mybir.dt.int32)
    fp8 = mybir.dt.float8e4

    # I/O
    xT = nc.dram_tensor("xT", [cfg.KPAD, cfg.SHP], bfl, kind="ExternalInput")
    gidx = nc.dram_tensor("gidx", [P, gcols], i16, kind="ExternalInput")
    dsel = nc.dram_tensor("dsel", [P, T * C], bfl, kind="ExternalInput")
    dinv_tbl = nc.dram_tensor("dinv_tbl", [P, cfg.MT], f32,
                              kind="ExternalInput")
    sqrow = nc.dram_tensor("sqrow", [1, T * P], bfl, kind="ExternalInput")
    vidx = nc.dram_tensor("vidx", [P, BCH2], i32, kind="ExternalInput")
    ohT = nc.dram_tensor("ohT", [OH, BMAX], bfl, kind="ExternalInput")
    wc1 = nc.dram_tensor("wc1", [cfg.KPAD, H], bfl, kind="ExternalInput")
    wc2 = nc.dram_tensor("wc2", [H, H], bfl, kind="ExternalInput")
    wh1 = nc.dram_tensor("wh1", [3 * P, HH], bfl, kind="ExternalInput")
    wh2 = nc.dram_tensor("wh2", [HH, HH // 2], bfl, kind="ExternalInput")
    wh3 = nc.dram_tensor("wh3", [HH // 2, 1], bfl, kind="ExternalInput")
    b1row = nc.dram_tensor("b1row", [1, H], bfl, kind="ExternalInput")
    b2row = nc.dram_tensor("b2row", [1, H], bfl, kind="ExternalInput")
    bh1v = nc.dram_tensor("bh1v", [HH, 1], f32, kind="ExternalInput")
    bh2v = nc.dram_tensor("bh2v", [HH // 2, 1], f32, kind="ExternalInput")
    bh3v = nc.dram_tensor("bh3v", [1, 1], f32, kind="ExternalInput")
    out = nc.dram_tensor("out", [1, BMAX], f32, kind="ExternalOutput")

    # internal DRAM
    z0in = nc.dram_tensor("z0in", [cfg.SHP, H], fp8, kind="Internal")
    z1in = nc.dram_tensor("z1in", [cfg.SHP, H], fp8, kind="Internal")
    h2in = nc.dram_tensor("h2in", [cfg.SHP, H], bfl, kind="Internal")
    Z0a = nc.dram_tensor("Z0a", [cfg.TBL, H], fp8, kind="Internal",
                         addr_space="Shared")
    Z0b = nc.dram_tensor("Z0b", [cfg.TBB, H], fp8, kind="Internal",
                         addr_space="Shared")
    Z1a = nc.dram_tensor("Z1a", [cfg.TBL, H], fp8, kind="Internal",
                         addr_space="Shared")
    Z1b = nc.dram_tensor("Z1b", [cfg.TBB, H], fp8, kind="Internal",
                         addr_space="Shared")
    rg = [list(range(NCORES))]

    with tile.TileContext(nc) as tc:
        with tc.tile_pool(name="const", bufs=1) as const:
            iota_i = const.tile([P, P], i32)
            nc.gpsimd.iota(iota_i[:], pattern=[[1, P]], base=0,
                           channel_multiplier=0)
            iota_b = const.tile([P, P], bfl)
            nc.vector.tensor_copy(iota_b[:], iota_i[:])
            ident = const.tile([P, P], bfl)
            make_identity(nc, ident[:])

            def load(ap, shape, dt):
                t = const.tile(shape, dt, tag=ap.tensor.name)
                nc.sync.dma_start(t[:], ap)
                return t

            wc1_sb = load(wc1.rearrange("(t p) n -> p t n", p=P)[:],
                          [P, cfg.KT, H], bfl)
            wc2_sb = load(wc2.rearrange("(t p) n -> p t n", p=P)[:],
                          [P, 2, H], bfl)
            wh1_sb = load(wh1.rearrange("(t p) n -> p t n", p=P)[:],
                          [P, 3, HH], bfl)
            wh2_sb = load(wh2[:], [HH, HH // 2], bfl)
            wh3_sb = load(wh3[:], [HH // 2, 1], bfl)
            b1_sb = load(b1row[:], [1, H], bfl)
            b2_sb = load(b2row[:], [1, H], bfl)
            bh1_sb = load(bh1v[:], [HH, 1], f32)
            bh2_sb = load(bh2v[:], [HH // 2, 1], f32)
            bh3_sb = load(bh3v[:], [1, 1], f32)
            gidx_sb = load(gidx[:], [P, gcols], i16)
            dsel_sb = load(dsel[:], [P, T * C], bfl)
            dinv_sb = load(dinv_tbl[:], [P, cfg.MT], f32)
            sqrow_sb = load(sqrow[:], [1, T * P], bfl)
            vidx_sb = load(vidx[:], [P, BCH2], i32)
            ohT_sb = load(ohT[:], [OH, BMAX], bfl)

            npad = cfg.SHP - cfg.T * P
            if npad > 0:
                zpad = const.tile([P, H], bfl)
                nc.any.memset(zpad[:], 0.0)
                zpad8 = const.tile([P, H], fp8)
                nc.any.memset(zpad8[:], 0.0)
                nc.sync.dma_start(z1in[cfg.T * P:cfg.SHP, :], zpad8[:npad, :])
                nc.sync.dma_start(h2in[cfg.T * P:cfg.SHP, :], zpad[:npad, :])

            # ---------------- phase A: conv1 z0 = dinv * (x @ Wc1)
            MBS = 7
            with tc.tile_pool(name="c1sb", bufs=3) as c1sb, \
                 tc.tile_pool(name="c1ev", bufs=3) as c1ev, \
                 tc.tile_pool(name="c1ps", bufs=MBS + 1, space="PSUM") as c1ps:
                for mb0 in range(0, cfg.MT, MBS):
                    mbn = min(MBS, cfg.MT - mb0)
                    accs = [c1ps.tile([P, H], f32, tag="convacc",
                                      name=f"convacc_{mb0}_{j}")
                            for j in range(mbn)]
                    for kt in range(cfg.KT):
                        slab = c1sb.tile([P, MBS * P], bfl, tag="slab")
                        nc.sync.dma_start(
                            slab[:, :mbn * P],
                            xT[kt * P:(kt + 1) * P, mb0 * P:(mb0 + mbn) * P])
                        for j in range(mbn):
                            nc.tensor.matmul(
                                accs[j][:], lhsT=slab[:, j * P:(j + 1) * P],
                                rhs=wc1_sb[:, kt, :],
                                start=(kt == 0), stop=(kt == cfg.KT - 1))
                    for j in range(mbn):
                        zb = c1ev.tile([P, H], fp8, tag="zev")
                        col = mb0 + j
                        nc.vector.tensor_scalar(
                            out=zb[:], in0=accs[j][:],
                            scalar1=dinv_sb[:, col:col + 1], scalar2=None,
                            op0=mybir.AluOpType.mult)
                        r0 = col * P
                        nc.sync.dma_start(z0in[r0:r0 + P, :], zb[:])
                    if mb0 + mbn >= cfg.HSH // P and mb0 < cfg.HSH // P:
                        nc.gpsimd.collective_compute(
                            "AllGather", mybir.AluOpType.bypass,
                            replica_groups=rg,
                            ins=[z0in[:cfg.HSH, :]], outs=[Z0a[:]])

            nc.gpsimd.collective_compute(
                "AllGather", mybir.AluOpType.bypass, replica_groups=rg,
                ins=[z0in[cfg.HSH:, :]], outs=[Z0b[:]])

            # ---------------- aggregation layers
            def agg_layer(Za, Zb, locin, b_sb, out_dram, do_conv2, lname,
                          mdt, odt, post_tile=None):
                with tc.tile_pool(name=f"agsb{lname}", bufs=6) as agsb, \
                     tc.tile_pool(name=f"agst{lname}", bufs=4) as agst, \
                     tc.tile_pool(name=f"agev{lname}", bufs=3) as agev, \
                     tc.tile_pool(name=f"agps{lname}", bufs=3,
                                  space="PSUM") as agps, \
                     tc.tile_pool(name=f"agp2{lname}", bufs=2,
                                  space="PSUM") as agp2:
                    qn = 0
                    for t in range(T):
                        ct = 1 + clo[t] + chi[t]
                        msg = agsb.tile([P, C, H], mdt, tag="msg")
                        # self chunk: contiguous local table rows
                        nc.sync.dma_start(msg[:, 0, :],
                                          locin[t * P:(t + 1) * P, :])
                        if nlo16[t] % P != 0 or nlo16[t] == 0:
                            nc.vector.memset(msg[:, 1 + nlo16[t] // P, :], 0.0)
                        if nlo16[t] > 0:
                            nc.gpsimd.dma_gather(
                                msg[:, 1:1 + clo[t], :], Za[:],
                                gidx_sb[:, lo_off_c[t]:lo_off_c[t + 1]],
                                nlo16[t], nlo16[t], H, single_packet=False,
                                queue_num=qn % NQ)
                            qn += 1
                        if nhi16[t] % P != 0 or nhi16[t] == 0:
                            nc.vector.memset(
                                msg[:, 1 + clo[t] + nhi16[t] // P, :], 0.0)
                        if nhi16[t] > 0:
                            nc.gpsimd.dma_gather(
                                msg[:, 1 + clo[t]:1 + clo[t] + chi[t], :],
                                Zb[:],
                                gidx_sb[:, hi_base + hi_off_c[t]:
                                        hi_base + hi_off_c[t + 1]],
                                nhi16[t], nhi16[t], H, single_packet=False,
                                queue_num=qn % NQ)
                            qn += 1
                        # one-hot build: st[p, c, j] = (dsel[p, c] == j)
                        st = agst.tile([P, C * P], mdt, tag="st")
                        dse = dsel_sb[:, t * C:t * C + ct]
                        dse_b = bass.AP(dse.tensor, dse.offset,
                                        [dse.ap[0], dse.ap[1], [0, P]])
                        io = iota_b[:]
                        io_b = bass.AP(io.tensor, io.offset,
                                       [io.ap[0], [0, ct], io.ap[1]])
                        ob = st[:, :ct * P].rearrange("p (c j) -> p c j", j=P)
                        nc.vector.tensor_tensor(out=ob, in0=io_b, in1=dse_b,
                                                op=mybir.AluOpType.is_equal)
                        acc = agps.tile([P, H], f32, tag="agacc")
                        nc.tensor.matmul(acc[:],
                                         lhsT=sqrow_sb[:, t * P:(t + 1) * P],
                                         rhs=b_sb[:], start=True, stop=False)
                        for ci in range(ct):
                            nc.tensor.matmul(acc[:],
                                             lhsT=st[:, ci * P:(ci + 1) * P],
                                             rhs=msg[:, ci, :],
                                             start=False,
                                             stop=(ci == ct - 1))
                        # h = relu(acc * dinv_d)
                        hb = agev.tile([P, H], bfl if do_conv2 else odt,
                                       tag="hb")
                        nc.vector.tensor_scalar(
                            out=hb[:], in0=acc[:],
                            scalar1=dinv_sb[:, t:t + 1], scalar2=0.0,
                            op0=mybir.AluOpType.mult,
                            op1=mybir.AluOpType.max)
                        if do_conv2:
                            ht = agev.tile([P, H], bfl, tag="ht")
                            for k in range(2):
                                pt = agp2.tile([P, P], bfl, space="PSUM",
                                               tag="pt")
                                nc.tensor.transpose(
                                    pt[:], hb[:, k * P:(k + 1) * P], ident[:])
                                nc.vector.tensor_copy(
                                    ht[:, k * P:(k + 1) * P], pt[:])
                            pz = agp2.tile([P, H], f32, tag="pz")
                            for k in range(2):
                                nc.tensor.matmul(
                                    pz[:], lhsT=ht[:, k * P:(k + 1) * P],
                                    rhs=wc2_sb[:, k, :],
                                    start=(k == 0), stop=(k == 1))
                            res = agev.tile([P, H], odt, tag="res")
                            nc.vector.tensor_scalar(
                                out=res[:], in0=pz[:],
                                scalar1=dinv_sb[:, t:t + 1], scalar2=None,
                                op0=mybir.AluOpType.mult)
                        else:
                            res = hb
                        nc.sync.dma_start(out_dram[t * P:(t + 1) * P, :],
                                          res[:])
                        if post_tile is not None:
                            post_tile(t)

            lo_off_c = [0]
            for t in range(T):
                lo_off_c.append(lo_off_c[-1] + nlo16[t] // 16)
            hi_off_c = [0]
            for t in range(T):
                hi_off_c.append(hi_off_c[-1] + nhi16[t] // 16)

            half_t = cfg.HSH // P - 1   # last tile of the A half

            def post1(t):
                if t == half_t:
                    nc.gpsimd.collective_compute(
                        "AllGather", mybir.AluOpType.bypass,
                        replica_groups=rg,
                        ins=[z1in[:cfg.HSH, :]], outs=[Z1a[:]])

            agg_layer(Z0a, Z0b, z0in, b1_sb, z1in, do_conv2=True, lname="a",
                      mdt=fp8, odt=fp8, post_tile=post1)
            nc.gpsimd.collective_compute(
                "AllGather", mybir.AluOpType.bypass, replica_groups=rg,
                ins=[z1in[cfg.HSH:, :]], outs=[Z1b[:]])

            agg_layer(Z1a, Z1b, z1in, b2_sb, h2in, do_conv2=False, lname="b",
                      mdt=fp8, odt=bfl, post_tile=None)

            # ---------------- head
            with tc.tile_pool(name="hdsb", bufs=2) as hdsb, \
                 tc.tile_pool(name="hdps", bufs=2, space="PSUM") as hdps:
                zt0 = hdsb.tile([P, BMAX], bfl, tag="zt0")
                zt1 = hdsb.tile([P, BMAX], bfl, tag="zt1")
                for j in range(BCH2):
                    g = hdsb.tile([P, H], bfl, tag="hg")
                    nc.gpsimd.indirect_dma_start(
                        out=g[:], out_offset=None, in_=h2in[:],
                        in_offset=bass.IndirectOffsetOnAxis(
                            ap=vidx_sb[:, j:j + 1], axis=0))
                    for k in range(2):
                        pt = hdps.tile([P, P], bfl, space="PSUM", tag="hpt")
                        nc.tensor.transpose(pt[:], g[:, k * P:(k + 1) * P],
                                            ident[:])
                        dstt = zt0 if k == 0 else zt1
                        nc.vector.tensor_copy(
                            dstt[:, j * P:(j + 1) * P], pt[:])
                for b0 in range(0, BMAX, 512):
                    bw = min(512, BMAX - b0)
                    ph1 = hdps.tile([P, 512], f32, tag="ph1")
                    nc.tensor.matmul(ph1[:, :bw], lhsT=wh1_sb[:, 0, :],
                                     rhs=zt0[:, b0:b0 + bw],
                                     start=True, stop=False)
                    nc.tensor.matmul(ph1[:, :bw], lhsT=wh1_sb[:, 1, :],
                                     rhs=zt1[:, b0:b0 + bw],
                                     start=False, stop=False)
                    nc.tensor.matmul(ph1[:, :bw], lhsT=wh1_sb[:OH, 2, :],
                                     rhs=ohT_sb[:, b0:b0 + bw],
                                     start=False, stop=True)
                    a1 = hdsb.tile([P, 512], bfl, tag="a1")
                    nc.scalar.activation(a1[:, :bw], ph1[:, :bw],
                                         mybir.ActivationFunctionType.Relu,
                                         bias=bh1_sb[:])
                    ph2 = hdps.tile([HH // 2, 512], f32, tag="ph2")
                    nc.tensor.matmul(ph2[:, :bw], lhsT=wh2_sb[:],
                                     rhs=a1[:, :bw], start=True, stop=True)
                    a2 = hdsb.tile([HH // 2, 512], bfl, tag="a2")
                    nc.scalar.activation(a2[:, :bw], ph2[:, :bw],
                                         mybir.ActivationFunctionType.Relu,
                                         bias=bh2_sb[:])
                    ph3 = hdps.tile([1, 512], f32, tag="ph3")
                    nc.tensor.matmul(ph3[:, :bw], lhsT=wh3_sb[:],
                                     rhs=a2[:, :bw], start=True, stop=True)
                    osb = hdsb.tile([1, 512], f32, tag="osb")
                    nc.vector.tensor_scalar_add(osb[:, :bw], ph3[:, :bw],
                                                bh3_sb[:, :1])
                    nc.sync.dma_start(out[:, b0:b0 + bw], osb[:, :bw])

    nc.compile()
    return nc


# ------------------------------------------------------------------ driver

_CACHE = {}


def _get_program(cfg, meta):
    key = (cfg.N, cfg.E, cfg.D_IN, cfg.B, meta["CL"], meta["CH"],
           meta["BMAX"], meta["nlo16"], meta["nhi16"])
    if key not in _CACHE:
        _CACHE[key] = build_program(cfg, meta)
    return _CACHE[key]


def assemble_out(cfg, meta, results):
    full = np.zeros(cfg.B, np.float32)
    for q in range(NCORES):
        pos = meta["positions"][q]
        vals = np.asarray(results[q]["out"]).reshape(meta["BMAX"])
        full[pos] = vals[:len(pos)]
    return full


def kernel(**inputs):
    cfg = REAL
    in_maps, meta = host_prep(cfg, **inputs)
    nc = _get_program(cfg, meta)
    from concourse import bass_utils
    res = bass_utils.run_bass_kernel_spmd(
        nc, in_maps, core_ids=list(range(NCORES)))
    return assemble_out(cfg, meta, res.results)


# revision 6
# speedup vs baseline: 1.7944x; 1.0835x over previous
"""Trainium2 Bass kernel for a 2-layer GCN encoder + MLP head (PyG GCNConv).

v2 strategy (8 NeuronCores, node-parallel):
  - Nodes sharded by contiguous range; dst tiles are contiguous 128-node
    blocks (T = ceil(SH/128)), so aggregation output writes are plain
    contiguous DMA (no indirect scatter).
  - norm = dinv[src]*dinv[dst] is factored: tables store dinv[i]*z[i]
    (scaled at conv evacuation); aggregation post-multiplies by dinv[d]
    fused with relu on DVE; bias enters via a 1-row matmul (sqdeg x b).
  - Self-loop edges form chunk 0 of each tile, loaded contiguously from the
    core-local table shard (z0in/z1in) with a plain dma_start.
  - Remaining edges: per tile two dma_gathers (lo/hi int16 index halves)
    with per-tile real counts (16-padded) and round-robin SWDGE queues
    (4 queues) to avoid descriptor-ring serialization.
  - One-hot routing matrices built on DVE with a single broadcast-AP
    is_equal per tile; aggregation = per-chunk one-hot matmuls in PSUM.
  - conv2 fused in the layer-1 tile loop via PE transposes.
All heavy compute bf16 with fp32 PSUM accumulation.
"""
import sys

for _p in ("/opt/trn_rl_repo",):
    if _p not in sys.path:
        sys.path.insert(0, _p)

import numpy as np
import ml_dtypes

bf16 = ml_dtypes.bfloat16

P = 128
H = 256          # gcn hidden width (fixed)
HH = 128         # head hidden width (fixed)
OH = 40          # wt_onehot + mut_onehot width (fixed)
NCORES = 8
NQ = 4           # SWDGE queues


class Cfg:
    def __init__(self, N, E, D_IN, B):
        self.N, self.E, self.D_IN, self.B = N, E, D_IN, B
        assert N % NCORES == 0
        self.SH = N // NCORES                      # real rows per shard
        self.T = -(-self.SH // P)                  # dst tiles per shard
        shp = max(self.T * P, -(-self.SH // P) * P)
        if shp == self.SH:
            shp += P
        self.SHP = -(-shp // P) * P                # padded rows per shard
        self.NP = NCORES * self.SHP                # padded global rows
        self.HSH = ((self.SHP // P + 1) // 2) * P  # A-half rows (P-aligned)
        self.HSB = self.SHP - self.HSH             # B-half rows
        self.TBL = NCORES * self.HSH               # A table rows
        self.TBB = NCORES * self.HSB               # B table rows
        assert self.TBL < 32768 and self.TBB < 32768
        self.KT = -(-D_IN // P)                    # k tiles for conv1
        self.KPAD = self.KT * P
        self.MT = self.SHP // P                    # m tiles per shard
        self.BPC = self.B // NCORES                # batch per core
        assert self.BPC % P == 0
        self.BCH = self.BPC // P                   # batch chunks


REAL = Cfg(N=50000, E=800000, D_IN=1281, B=4096)


# ---------------------------------------------------------------- host prep

def _pack_idx16(seq):
    """idx sequence [n] (n%16==0) -> wrapped-16 replicated [128, n//16] i16."""
    n = seq.shape[0]
    assert n % 16 == 0
    a = seq.reshape(n // 16, 16).T.astype(np.int16)
    return np.tile(a, (8, 1))


def host_prep(cfg, x, wt_onehot, mut_onehot, Wc1, bc1, Wc2, bc2,
              Wh1, bh1, Wh2, bh2, Wh3, bh3, edge_index, var_node_idx):
    N, E, SH, SHP, T = cfg.N, cfg.E, cfg.SH, cfg.SHP, cfg.T
    src = np.asarray(edge_index[0], np.int64)
    dst = np.asarray(edge_index[1], np.int64)
    # degree includes self loops (dst counts + 1)
    deg = (np.bincount(dst, minlength=N) + 1).astype(np.float32)
    dinv = (1.0 / np.sqrt(deg)).astype(np.float32)
    sqdeg = np.sqrt(deg).astype(np.float32)
    q_of = src // SH
    r_of = src % SH
    in_b = r_of >= cfg.HSH
    srcp = np.where(in_b, q_of * cfg.HSB + (r_of - cfg.HSH),
                    q_of * cfg.HSH + r_of)

    # drop self-referencing edges? (none in random data, but (i,i) entries in
    # edge_index are real edges distinct from the implicit self loop)
    core_of = dst // SH

    # per-core, per-tile edge lists
    per_core = []
    CLmax = CHmax = 0
    for q in range(NCORES):
        m = core_of == q
        d_loc = dst[m] - q * SH
        sp = srcp[m]
        tile_of = d_loc // P
        order = np.argsort(tile_of, kind="stable")
        d_s, sp_s, t_s = d_loc[order], sp[order], tile_of[order]
        tstarts = np.searchsorted(t_s, np.arange(T + 1))
        tiles = []
        inb_s = in_b[m][order]
        for t in range(T):
            a, b = tstarts[t], tstarts[t + 1]
            spt, dt_, ib = sp_s[a:b], d_s[a:b] - t * P, inb_s[a:b]
            lo = ~ib
            tiles.append(((spt[lo], dt_[lo]), (spt[ib], dt_[ib])))
            CLmax = max(CLmax, -(-int(lo.sum()) // P))
            CHmax = max(CHmax, -(-int(ib.sum()) // P))
        per_core.append(tiles)
    CL, CH = max(1, int(CLmax)), max(1, int(CHmax))
    C = 1 + CL + CH   # self chunk + lo chunks + hi chunks

    # per-tile static counts must be IDENTICAL across cores (one program):
    # use the max over cores for each tile's lo/hi counts.
    nlo = np.zeros(T, np.int64)
    nhi = np.zeros(T, np.int64)
    for q in range(NCORES):
        for t in range(T):
            (sl, _), (sh_, _) = per_core[q][t]
            nlo[t] = max(nlo[t], len(sl))
            nhi[t] = max(nhi[t], len(sh_))
    nlo16 = ((nlo + 15) // 16) * 16
    nhi16 = ((nhi + 15) // 16) * 16
    clo = np.maximum(1, -(-nlo16 // P))  # chunks per tile (>=1 for layout)
    chi = np.maximum(1, -(-nhi16 // P))
    # column offsets into gidx (units of 16-idx columns)
    lo_off = np.zeros(T + 1, np.int64)
    hi_off = np.zeros(T + 1, np.int64)
    np.cumsum(nlo16 // 16, out=lo_off[1:])
    hi_base = lo_off[T]
    np.cumsum(nhi16 // 16, out=hi_off[1:])
    gcols = int(lo_off[T] + hi_off[T])

    vni = np.asarray(var_node_idx, np.int64)
    vq, vr = vni // SH, vni % SH
    positions = [np.nonzero(vq == q)[0] for q in range(NCORES)]
    bmax = max(len(p) for p in positions)
    BMAX = ((bmax + P - 1) // P) * P

    meta = dict(CL=CL, CH=CH, C=C, BMAX=BMAX, positions=positions,
                nlo16=tuple(int(v) for v in nlo16),
                nhi16=tuple(int(v) for v in nhi16),
                clo=tuple(int(v) for v in clo),
                chi=tuple(int(v) for v in chi),
                gcols=gcols, hi_base=int(hi_base))

    # shared weights
    wc1 = np.zeros((cfg.KPAD, H), bf16)
    wc1[:cfg.D_IN] = np.asarray(Wc1, np.float32).astype(bf16)
    wc2 = np.asarray(Wc2, np.float32).astype(bf16)
    wh1 = np.zeros((3 * P, HH), bf16)
    wh1[:H + OH] = np.asarray(Wh1, np.float32).astype(bf16)
    wh2 = np.asarray(Wh2, np.float32).astype(bf16)
    wh3 = np.asarray(Wh3, np.float32).astype(bf16)
    b1row = np.asarray(bc1, np.float32).astype(bf16).reshape(1, H)
    b2row = np.asarray(bc2, np.float32).astype(bf16).reshape(1, H)
    bh1v = np.asarray(bh1, np.float32).reshape(HH, 1)
    bh2v = np.asarray(bh2, np.float32).reshape(HH // 2, 1)
    bh3v = np.asarray(bh3, np.float32).reshape(1, 1)

    x = np.asarray(x, np.float32)
    wt_b = np.asarray(wt_onehot, np.float32).astype(bf16)
    mut_b = np.asarray(mut_onehot, np.float32).astype(bf16)

    in_maps = []
    for q in range(NCORES):
        gidx_seq = np.zeros(gcols * 16, np.int64)
        dsel = np.full((P, T * C), 999.0, np.float32)
        for t in range(T):
            (sl, dl), (sh_, dh_) = per_core[q][t]
            # self chunk col: diagonal for real rows
            nreal = min(P, SH - t * P)
            dsel[:nreal, t * C] = np.arange(nreal, dtype=np.float32)
            base = lo_off[t] * 16
            gidx_seq[base:base + len(sl)] = sl
            a_d = np.full(clo[t] * P, 999.0, np.float32)
            a_d[:len(dl)] = dl
            dsel[:, t * C + 1:t * C + 1 + clo[t]] = \
                a_d.reshape(clo[t], P).T
            base = (hi_base + hi_off[t]) * 16
            gidx_seq[base:base + len(sh_)] = sh_
            a_d = np.full(chi[t] * P, 999.0, np.float32)
            a_d[:len(dh_)] = dh_
            dsel[:, t * C + 1 + clo[t]:t * C + 1 + clo[t] + chi[t]] = \
                a_d.reshape(chi[t], P).T
        gidx = _pack_idx16(gidx_seq)

        # dinv per m-tile column [128, MT]; sqdeg row [1, T*P]
        dloc = np.zeros(SHP, np.float32)
        dloc[:SH] = dinv[q * SH:(q + 1) * SH]
        dinv_tbl = dloc.reshape(cfg.MT, P).T.copy()
        sq = np.zeros(T * P, np.float32)
        sq[:SH] = sqdeg[q * SH:(q + 1) * SH]
        sqrow = sq.reshape(1, T * P).astype(bf16)

        xT = np.zeros((cfg.KPAD, SHP), bf16)
        xT[:cfg.D_IN, :SH] = x[q * SH:(q + 1) * SH].T.astype(bf16)
        pos = positions[q]
        vloc = np.zeros(BMAX, np.int64)
        vloc[:len(pos)] = vr[pos]
        vidx = vloc.reshape(BMAX // P, P).T.astype(np.int32)
        ohp = np.zeros((BMAX, 2 * 20), bf16)
        ohp[:len(pos), :20] = wt_b[pos]
        ohp[:len(pos), 20:] = mut_b[pos]
        ohT = ohp.T.copy()  # [40, BMAX]
        in_maps.append(dict(
            xT=xT, gidx=gidx, dsel=dsel.astype(bf16),
            dinv_tbl=np.ascontiguousarray(dinv_tbl),
            sqrow=np.ascontiguousarray(sqrow),
            vidx=np.ascontiguousarray(vidx), ohT=np.ascontiguousarray(ohT),
            wc1=wc1, wc2=wc2, wh1=wh1, wh2=wh2, wh3=wh3,
            b1row=b1row, b2row=b2row,
            bh1v=bh1v, bh2v=bh2v, bh3v=bh3v,
        ))
    return in_maps, meta


# ------------------------------------------------------------- bass program

def build_program(cfg, meta):
    import concourse.bass as bass
    import concourse.mybir as mybir
    import concourse.tile as tile
    from concourse import bacc
    from concourse.masks import make_identity

    T = cfg.T
    CL, CH, C = meta["CL"], meta["CH"], meta["C"]
    nlo16, nhi16 = meta["nlo16"], meta["nhi16"]
    clo, chi = meta["clo"], meta["chi"]
    gcols, hi_base = meta["gcols"], meta["hi_base"]
    BMAX = meta["BMAX"]
    BCH2 = BMAX // P

    nc = bacc.Bacc("TRN2", target_bir_lowering=False, debug=False,
                   num_devices=NCORES, num_swdge_queues=NQ,
                   dynamic_dma_scratch_size=49152)
    f32, bfl, i16, i32 = (mybir.dt.float32, mybir.dt.bfloat16,
                          mybir.dt.int16, mybir.dt.int32)
    fp8 = mybir.dt.float8e4

    # I/O
    xT = nc.dram_tensor("xT", [cfg.KPAD, cfg.SHP], bfl, kind="ExternalInput")
    gidx = nc.dram_tensor("gidx", [P, gcols], i16, kind="ExternalInput")
    dsel = nc.dram_tensor("dsel", [P, T * C], bfl, kind="ExternalInput")
    dinv_tbl = nc.dram_tensor("dinv_tbl", [P, cfg.MT], f32,
                              kind="ExternalInput")
    sqrow = nc.dram_tensor("sqrow", [1, T * P], bfl, kind="ExternalInput")
    vidx = nc.dram_tensor("vidx", [P, BCH2], i32, kind="ExternalInput")
    ohT = nc.dram_tensor("ohT", [OH, BMAX], bfl, kind="ExternalInput")
    wc1 = nc.dram_tensor("wc1", [cfg.KPAD, H], bfl, kind="ExternalInput")
    wc2 = nc.dram_tensor("wc2", [H, H], bfl, kind="ExternalInput")
    wh1 = nc.dram_tensor("wh1", [3 * P, HH], bfl, kind="ExternalInput")
    wh2 = nc.dram_tensor("wh2", [HH, HH // 2], bfl, kind="ExternalInput")
    wh3 = nc.dram_tensor("wh3", [HH // 2, 1], bfl, kind="ExternalInput")
    b1row = nc.dram_tensor("b1row", [1, H], bfl, kind="ExternalInput")
    b2row = nc.dram_tensor("b2row", [1, H], bfl, kind="ExternalInput")
    bh1v = nc.dram_tensor("bh1v", [HH, 1], f32, kind="ExternalInput")
    bh2v = nc.dram_tensor("bh2v", [HH // 2, 1], f32, kind="ExternalInput")
    bh3v = nc.dram_tensor("bh3v", [1, 1], f32, kind="ExternalInput")
    out = nc.dram_tensor("out", [1, BMAX], f32, kind="ExternalOutput")

    # internal DRAM
    z0in = nc.dram_tensor("z0in", [cfg.SHP, H], fp8, kind="Internal")
    z1in = nc.dram_tensor("z1in", [cfg.SHP, H], fp8, kind="Internal")
    h2in = nc.dram_tensor("h2in", [cfg.SHP, H], bfl, kind="Internal")
    Z0a = nc.dram_tensor("Z0a", [cfg.TBL, H], fp8, kind="Internal",
                         addr_space="Shared")
    Z0b = nc.dram_tensor("Z0b", [cfg.TBB, H], fp8, kind="Internal",
                         addr_space="Shared")
    Z1a = nc.dram_tensor("Z1a", [cfg.TBL, H], fp8, kind="Internal",
                         addr_space="Shared")
    Z1b = nc.dram_tensor("Z1b", [cfg.TBB, H], fp8, kind="Internal",
                         addr_space="Shared")
    rg = [list(range(NCORES))]

    with tile.TileContext(nc) as tc:
        with tc.tile_pool(name="const", bufs=1) as const:
            iota_i = const.tile([P, P], i32)
            nc.gpsimd.iota(iota_i[:], pattern=[[1, P]], base=0,
                           channel_multiplier=0)
            iota_b = const.tile([P, P], bfl)
            nc.vector.tensor_copy(iota_b[:], iota_i[:])
            ident = const.tile([P, P], bfl)
            make_identity(nc, ident[:])

            def load(ap, shape, dt):
                t = const.tile(shape, dt, tag=ap.tensor.name)
                nc.sync.dma_start(t[:], ap)
                return t

            wc1_sb = load(wc1.rearrange("(t p) n -> p t n", p=P)[:],
                          [P, cfg.KT, H], bfl)
            wc2_sb = load(wc2.rearrange("(t p) n -> p t n", p=P)[:],
                          [P, 2, H], bfl)
            wh1_sb = load(wh1.rearrange("(t p) n -> p t n", p=P)[:],
                          [P, 3, HH], bfl)
            wh2_sb = load(wh2[:], [HH, HH // 2], bfl)
            wh3_sb = load(wh3[:], [HH // 2, 1], bfl)
            b1_sb = load(b1row[:], [1, H], bfl)
            b2_sb = load(b2row[:], [1, H], bfl)
            bh1_sb = load(bh1v[:], [HH, 1], f32)
            bh2_sb = load(bh2v[:], [HH // 2, 1], f32)
            bh3_sb = load(bh3v[:], [1, 1], f32)
            gidx_sb = load(gidx[:], [P, gcols], i16)
            dsel_sb = load(dsel[:], [P, T * C], bfl)
            dinv_sb = load(dinv_tbl[:], [P, cfg.MT], f32)
            sqrow_sb = load(sqrow[:], [1, T * P], bfl)
            vidx_sb = load(vidx[:], [P, BCH2], i32)
            ohT_sb = load(ohT[:], [OH, BMAX], bfl)

            npad = cfg.SHP - cfg.T * P
            if npad > 0:
                zpad = const.tile([P, H], bfl)
                nc.any.memset(zpad[:], 0.0)
                zpad8 = const.tile([P, H], fp8)
                nc.any.memset(zpad8[:], 0.0)
                nc.sync.dma_start(z1in[cfg.T * P:cfg.SHP, :], zpad8[:npad, :])
                nc.sync.dma_start(h2in[cfg.T * P:cfg.SHP, :], zpad[:npad, :])

            # ---------------- phase A: conv1 z0 = dinv * (x @ Wc1)
            MBS = 7
            with tc.tile_pool(name="c1sb", bufs=3) as c1sb, \
                 tc.tile_pool(name="c1ev", bufs=3) as c1ev, \
                 tc.tile_pool(name="c1ps", bufs=MBS + 1, space="PSUM") as c1ps:
                for mb0 in range(0, cfg.MT, MBS):
                    mbn = min(MBS, cfg.MT - mb0)
                    accs = [c1ps.tile([P, H], f32, tag="convacc",
                                      name=f"convacc_{mb0}_{j}")
                            for j in range(mbn)]
                    for kt in range(cfg.KT):
                        slab = c1sb.tile([P, MBS * P], bfl, tag="slab")
                        nc.sync.dma_start(
                            slab[:, :mbn * P],
                            xT[kt * P:(kt + 1) * P, mb0 * P:(mb0 + mbn) * P])
                        for j in range(mbn):
                            nc.tensor.matmul(
                                accs[j][:], lhsT=slab[:, j * P:(j + 1) * P],
                                rhs=wc1_sb[:, kt, :],
                                start=(kt == 0), stop=(kt == cfg.KT - 1))
                    for j in range(mbn):
                        zb = c1ev.tile([P, H], fp8, tag="zev")
                        col = mb0 + j
                        nc.vector.tensor_scalar(
                            out=zb[:], in0=accs[j][:],
                            scalar1=dinv_sb[:, col:col + 1], scalar2=None,
                            op0=mybir.AluOpType.mult)
                        r0 = col * P
                        nc.sync.dma_start(z0in[r0:r0 + P, :], zb[:])
                    if mb0 + mbn >= cfg.HSH // P and mb0 < cfg.HSH // P:
                        nc.gpsimd.collective_compute(
                            "AllGather", mybir.AluOpType.bypass,
                            replica_groups=rg,
                            ins=[z0in[:cfg.HSH, :]], outs=[Z0a[:]])

            nc.gpsimd.collective_compute(
                "AllGather", mybir.AluOpType.bypass, replica_groups=rg,
                ins=[z0in[cfg.HSH:, :]], outs=[Z0b[:]])

            # ---------------- aggregation layers
            def agg_layer(Za, Zb, locin, b_sb, out_dram, do_conv2, lname,
                          mdt, odt, post_tile=None):
                with tc.tile_pool(name=f"agsb{lname}", bufs=8) as agsb, \
                     tc.tile_pool(name=f"agst{lname}", bufs=6) as agst, \
                     tc.tile_pool(name=f"agev{lname}", bufs=3) as agev, \
                     tc.tile_pool(name=f"agps{lname}", bufs=3,
                                  space="PSUM") as agps, \
                     tc.tile_pool(name=f"agp2{lname}", bufs=2,
                                  space="PSUM") as agp2:
                    qn = 0
                    for t in range(T):
                        ct = 1 + clo[t] + chi[t]
                        msg = agsb.tile([P, C, H], mdt, tag="msg")
                        # self chunk: contiguous local table rows
                        nc.sync.dma_start(msg[:, 0, :],
                                          locin[t * P:(t + 1) * P, :])
                        if nlo16[t] % P != 0 or nlo16[t] == 0:
                            nc.vector.memset(msg[:, 1 + nlo16[t] // P, :], 0.0)
                        if nlo16[t] > 0:
                            nc.gpsimd.dma_gather(
                                msg[:, 1:1 + clo[t], :], Za[:],
                                gidx_sb[:, lo_off_c[t]:lo_off_c[t + 1]],
                                nlo16[t], nlo16[t], H, single_packet=False,
                                queue_num=qn % NQ)
                            qn += 1
                        if nhi16[t] % P != 0 or nhi16[t] == 0:
                            nc.vector.memset(
                                msg[:, 1 + clo[t] + nhi16[t] // P, :], 0.0)
                        if nhi16[t] > 0:
                            nc.gpsimd.dma_gather(
                                msg[:, 1 + clo[t]:1 + clo[t] + chi[t], :],
                                Zb[:],
                                gidx_sb[:, hi_base + hi_off_c[t]:
                                        hi_base + hi_off_c[t + 1]],
                                nhi16[t], nhi16[t], H, single_packet=False,
                                queue_num=qn % NQ)
                            qn += 1
                        # one-hot build: st[p, c, j] = (dsel[p, c] == j)
                        st = agst.tile([P, C * P], mdt, tag="st")
                        dse = dsel_sb[:, t * C:t * C + ct]
                        dse_b = bass.AP(dse.tensor, dse.offset,
                                        [dse.ap[0], dse.ap[1], [0, P]])
                        io = iota_b[:]
                        io_b = bass.AP(io.tensor, io.offset,
                                       [io.ap[0], [0, ct], io.ap[1]])
                        ob = st[:, :ct * P].rearrange("p (c j) -> p c j", j=P)
                        nc.vector.tensor_tensor(out=ob, in0=io_b, in1=dse_b,
                                                op=mybir.AluOpType.is_equal)
                        acc = agps.tile([P, H], f32, tag="agacc")
                        nc.tensor.matmul(acc[:],
                                         lhsT=sqrow_sb[:, t * P:(t + 1) * P],
                                         rhs=b_sb[:], start=True, stop=False)
                        for ci in range(ct):
                            nc.tensor.matmul(acc[:],
                                             lhsT=st[:, ci * P:(ci + 1) * P],
                                             rhs=msg[:, ci, :],
                                             start=False,
                                             stop=(ci == ct - 1))
                        # h = relu(acc * dinv_d)
                        hb = agev.tile([P, H], bfl if do_conv2 else odt,
                                       tag="hb")
                        nc.vector.tensor_scalar(
                            out=hb[:], in0=acc[:],
                            scalar1=dinv_sb[:, t:t + 1], scalar2=0.0,
                            op0=mybir.AluOpType.mult,
                            op1=mybir.AluOpType.max)
                        if do_conv2:
                            ht = agev.tile([P, H], bfl, tag="ht")
                            for k in range(2):
                                pt = agp2.tile([P, P], bfl, space="PSUM",
                                               tag="pt")
                                nc.tensor.transpose(
                                    pt[:], hb[:, k * P:(k + 1) * P], ident[:])
                                nc.vector.tensor_copy(
                                    ht[:, k * P:(k + 1) * P], pt[:])
                            pz = agp2.tile([P, H], f32, tag="pz")
                            for k in range(2):
                                nc.tensor.matmul(
                                    pz[:], lhsT=ht[:, k * P:(k + 1) * P],
                                    rhs=wc2_sb[:, k, :],
                                    start=(k == 0), stop=(k == 1))
                            res = agev.tile([P, H], odt, tag="res")
                            nc.vector.tensor_scalar(
                                out=res[:], in0=pz[:],
                                scalar1=dinv_sb[:, t:t + 1], scalar2=None,
                                op0=mybir.AluOpType.mult)
                        else:
                            res = hb
                        nc.sync.dma_start(out_dram[t * P:(t + 1) * P, :],
                                          res[:])
                        if post_tile is not None:
                            post_tile(t)

            lo_off_c = [0]
            for t in range(T):
                lo_off_c.append(lo_off_c[-1] + nlo16[t] // 16)
            hi_off_c = [0]
            for t in range(T):
                hi_off_c.append(hi_off_c[-1] + nhi16[t] // 16)

            half_t = cfg.HSH // P - 1   # last tile of the A half

            def post1(t):
                if t == half_t:
                    nc.gpsimd.collective_compute(
                        "AllGather", mybir.AluOpType.bypass,
                        replica_groups=rg,
                        ins=[z1in[:cfg.HSH, :]], outs=[Z1a[:]])

            agg_layer(Z0a, Z0b, z0in, b1_sb, z1in, do_conv2=True, lname="a",
                      mdt=fp8, odt=fp8, post_tile=post1)
            nc.gpsimd.collective_compute(
                "AllGather", mybir.AluOpType.bypass, replica_groups=rg,
                ins=[z1in[cfg.HSH:, :]], outs=[Z1b[:]])

            agg_layer(Z1a, Z1b, z1in, b2_sb, h2in, do_conv2=False, lname="b",
                      mdt=fp8, odt=bfl, post_tile=None)

            # ---------------- head
            with tc.tile_pool(name="hdsb", bufs=2) as hdsb, \
                 tc.tile_pool(name="hdps", bufs=2, space="PSUM") as hdps:
                zt0 = hdsb.tile([P, BMAX], bfl, tag="zt0")
                zt1 = hdsb.tile([P, BMAX], bfl, tag="zt1")
                for j in range(BCH2):
                    g = hdsb.tile([P, H], bfl, tag="hg")
                    nc.gpsimd.indirect_dma_start(
                        out=g[:], out_offset=None, in_=h2in[:],
                        in_offset=bass.IndirectOffsetOnAxis(
                            ap=vidx_sb[:, j:j + 1], axis=0))
                    for k in range(2):
                        pt = hdps.tile([P, P], bfl, space="PSUM", tag="hpt")
                        nc.tensor.transpose(pt[:], g[:, k * P:(k + 1) * P],
                                            ident[:])
                        dstt = zt0 if k == 0 else zt1
                        nc.vector.tensor_copy(
                            dstt[:, j * P:(j + 1) * P], pt[:])
                for b0 in range(0, BMAX, 512):
                    bw = min(512, BMAX - b0)
                    ph1 = hdps.tile([P, 512], f32, tag="ph1")
                    nc.tensor.matmul(ph1[:, :bw], lhsT=wh1_sb[:, 0, :],
                                     rhs=zt0[:, b0:b0 + bw],
                                     start=True, stop=False)
                    nc.tensor.matmul(ph1[:, :bw], lhsT=wh1_sb[:, 1, :],
                                     rhs=zt1[:, b0:b0 + bw],
                                     start=False, stop=False)
                    nc.tensor.matmul(ph1[:, :bw], lhsT=wh1_sb[:OH, 2, :],
                                     rhs=ohT_sb[:, b0:b0 + bw],
                                     start=False, stop=True)
                    a1 = hdsb.tile([P, 512], bfl, tag="a1")
                    nc.scalar.activation(a1[:, :bw], ph1[:, :bw],
                                         mybir.ActivationFunctionType.Relu,
                                         bias=bh1_sb[:])
                    ph2 = hdps.tile([HH // 2, 512], f32, tag="ph2")
                    nc.tensor.matmul(ph2[:, :bw], lhsT=wh2_sb[:],
                                     rhs=a1[:, :bw], start=True, stop=True)
                    a2 = hdsb.tile([HH // 2, 512], bfl, tag="a2")
                    nc.scalar.activation(a2[:, :bw], ph2[:, :bw],
                                         mybir.ActivationFunctionType.Relu,
                                         bias=bh2_sb[:])
                    ph3 = hdps.tile([1, 512], f32, tag="ph3")
                    nc.tensor.matmul(ph3[:, :bw], lhsT=wh3_sb[:],
                                     rhs=a2[:, :bw], start=True, stop=True)
                    osb = hdsb.tile([1, 512], f32, tag="osb")
                    nc.vector.tensor_scalar_add(osb[:, :bw], ph3[:, :bw],
                                                bh3_sb[:, :1])
                    nc.sync.dma_start(out[:, b0:b0 + bw], osb[:, :bw])

    nc.compile()
    return nc


# ------------------------------------------------------------------ driver

_CACHE = {}


def _get_program(cfg, meta):
    key = (cfg.N, cfg.E, cfg.D_IN, cfg.B, meta["CL"], meta["CH"],
           meta["BMAX"], meta["nlo16"], meta["nhi16"])
    if key not in _CACHE:
        _CACHE[key] = build_program(cfg, meta)
    return _CACHE[key]


def assemble_out(cfg, meta, results):
    full = np.zeros(cfg.B, np.float32)
    for q in range(NCORES):
        pos = meta["positions"][q]
        vals = np.asarray(results[q]["out"]).reshape(meta["BMAX"])
        full[pos] = vals[:len(pos)]
    return full


def kernel(**inputs):
    cfg = REAL
    in_maps, meta = host_prep(cfg, **inputs)
    nc = _get_program(cfg, meta)
    from concourse import bass_utils
    res = bass_utils.run_bass_kernel_spmd(
        nc, in_maps, core_ids=list(range(NCORES)))
    return assemble_out(cfg, meta, res.results)
